# revision 1
# baseline (speedup 1.0000x reference)
"""Trainium2 Bass kernel for conv-projected multi-head attention.

Reference computation (per batch item b of 8, one NeuronCore each):
  y   = BN(depthwise3x3(x_b reshaped to [C,32,32]))      # q = k = v = y
  q/k/v = y @ w{q,k,v}^T  (heads: 12 x 32)
  att = softmax((q @ k^T) * sqrt(32))
  out = (att @ v) @ wo^T

Device layout is channel-major ("transposed"): xT [C=384, T=1024].
 - conv: 9 accumulating diag-matmuls on PE over a zero-padded [34x34] image
 - qT/kT [o, t] via lhsT=w^T; v kept [t, o] (augmented with a ones column
   so the PV matmul also yields the softmax denominators)
 - scores S^T[t, l] per head with K=32 contraction, 4 heads row-packed in
   the PE array via tile_position
 - exp on ACT directly from a 4-bank PSUM tile, output fp32r
 - PV: lhsT = vaug [t,34], rhs = E [t,l]; out rows 0-31 = O^T, row 32 = sums
 - normalize after PV (per-head reciprocal broadcast), concat, out-proj
All matmuls in float32r (~2e-4 rel err, full PE rate).
"""
import sys

sys.path.insert(0, "/opt/trn_rl_repo")
from contextlib import ExitStack

import numpy as np

B, T, C = 8, 1024, 384
NH, DH = 12, 32
HH = WW = 32
SCALE = float(DH) ** 0.5
BN_EPS = 1e-5
NCORES = 8

_CACHE = {}


def _build(debug=False, stage=5):
    import concourse.bass as bass
    import concourse.tile as tile
    from concourse import bacc, mybir
    from concourse.masks import make_identity

    F32 = mybir.dt.float32
    F32R = mybir.dt.float32r
    AF = mybir.ActivationFunctionType
    ALU = mybir.AluOpType

    nc = bacc.Bacc("TRN2", target_bir_lowering=False, debug=False)

    xt_d = nc.dram_tensor("xt", [C, T], F32R, kind="ExternalInput").ap()
    w9_d = nc.dram_tensor("w9", [C, 9], F32, kind="ExternalInput").ap()
    bias_d = nc.dram_tensor("bias", [C, 1], F32, kind="ExternalInput").ap()
    wqT_d = nc.dram_tensor("wqT", [C, C], F32R, kind="ExternalInput").ap()
    wkT_d = nc.dram_tensor("wkT", [C, C], F32R, kind="ExternalInput").ap()
    wvT_d = nc.dram_tensor("wvT", [C, C], F32R, kind="ExternalInput").ap()
    woT_d = nc.dram_tensor("woT", [C, C], F32R, kind="ExternalInput").ap()
    ind_d = nc.dram_tensor("ind", [4, 34, 128], F32R, kind="ExternalInput").ap()
    bind_d = nc.dram_tensor("bind", [4, 128], F32R, kind="ExternalInput").ap()
    outT_d = nc.dram_tensor("outT", [C, T], F32, kind="ExternalOutput").ap()
    dbg = {}
    if debug:
        dbg["y"] = nc.dram_tensor("dbg_y", [C, T], F32, kind="ExternalOutput").ap()
        dbg["qT"] = nc.dram_tensor("dbg_qT", [C, T], F32, kind="ExternalOutput").ap()
        dbg["vaug"] = nc.dram_tensor(
            "dbg_vaug", [T, NH * 34], F32, kind="ExternalOutput"
        ).ap()
        dbg["E0"] = nc.dram_tensor(
            "dbg_E0", [2, T, 4 * 512], F32, kind="ExternalOutput"
        ).ap()
        dbg["attn"] = nc.dram_tensor(
            "dbg_attn", [C, T], F32, kind="ExternalOutput"
        ).ap()
        dbg["ov"] = nc.dram_tensor(
            "dbg_ov", [4, 34, T], F32, kind="ExternalOutput"
        ).ap()
        dbg["R"] = nc.dram_tensor("dbg_R", [128, T], F32, kind="ExternalOutput").ap()

    CT = C // 128  # 3 c-tiles
    TT = T // 128  # 8 t-tiles
    TH = T // 512  # 2 t-halves / l-halves

    with tile.TileContext(nc) as tc, ExitStack() as top:
        # ---- persistent pools ----
        persist = top.enter_context(tc.tile_pool(name="persist", bufs=1))
        copies = top.enter_context(tc.tile_pool(name="copies", bufs=3))

        # persistent SBUF tensors
        y_sb = [persist.tile([128, T], F32R, tag=f"y{i}", name=f"y{i}") for i in range(CT)]
        qT_sb = [persist.tile([128, T], F32R, tag=f"q{i}", name=f"q{i}") for i in range(CT)]
        kT_sb = [persist.tile([128, T], F32R, tag=f"k{i}", name=f"k{i}") for i in range(CT)]
        vaug = [persist.tile([128, NH, 34], F32R, tag=f"va{i}", name=f"va{i}") for i in range(TT)]
        attn_sb = [persist.tile([128, T], F32R, tag=f"at{i}", name=f"at{i}") for i in range(CT)] if stage >= 4 else None

        with ExitStack() as ph1:
            convpool = ph1.enter_context(tc.tile_pool(name="convpool", bufs=1))
            conv_ps = ph1.enter_context(
                tc.tile_pool(name="conv_ps", bufs=2, space="PSUM")
            )
            qk_ps = ph1.enter_context(tc.tile_pool(name="qk_ps", bufs=4, space="PSUM"))
            v_ps = ph1.enter_context(tc.tile_pool(name="v_ps", bufs=2, space="PSUM"))

            # ---- padded input and diag weights ----
            # x is DMA'd contiguously (fast), then the zero-padded 34x34
            # image buffer is built with DVE copies (strided SBUF writes).
            xt_sb = [convpool.tile([128, T], F32R, tag=f"xt{i}", name=f"xt{i}") for i in range(CT)]
            xp = [convpool.tile([128, 34 * 34], F32R, tag=f"xp{i}", name=f"xp{i}") for i in range(CT)]
            w9_sb = [convpool.tile([128, 9], F32, tag=f"w9{i}", name=f"w9s{i}") for i in range(CT)]
            ident = convpool.tile([128, 128], F32, tag="ident")
            diag = [convpool.tile([128, 9, 128], F32R, tag=f"dg{i}", name=f"dg{i}") for i in range(CT)]

            make_identity(nc, ident[:])
            for i in range(CT):
                nc.sync.dma_start(xt_sb[i][:], xt_d[i * 128 : (i + 1) * 128, :])
                nc.sync.dma_start(w9_sb[i][:], w9_d[i * 128 : (i + 1) * 128, :])
                nc.vector.memset(xp[i][:].bitcast(F32), 0.0)
                nc.vector.tensor_copy(
                    xp[i][:].rearrange("p (a b) -> p a b", a=34)[:, 1:33, 1:33],
                    xt_sb[i][:].rearrange("p (a b) -> p a b", a=32),
                )
                for k in range(9):
                    nc.vector.tensor_scalar_mul(
                        diag[i][:, k, :], ident[:], w9_sb[i][:, k : k + 1]
                    )

            # weight / constant DMAs are emitted after the conv inputs so the
            # DMA queue delivers xt first and conv starts early
            wT_sb = {}
            for nm, d in (("q", wqT_d), ("k", wkT_d), ("v", wvT_d), ("o", woT_d)):
                tiles = [persist.tile([128, C], F32R, tag=f"w{nm}{i}", name=f"w{nm}{i}") for i in range(CT)]
                for i in range(CT):
                    nc.sync.dma_start(tiles[i][:], d[i * 128 : (i + 1) * 128, :])
                wT_sb[nm] = tiles

            bias_sb = [persist.tile([128, 1], F32, tag=f"b{i}", name=f"b{i}") for i in range(CT)]
            for i in range(CT):
                nc.sync.dma_start(bias_sb[i][:], bias_d[i * 128 : (i + 1) * 128, :])

            # ---- conv: 9 accumulating diag matmuls per (c-tile, t-half) ----
            for i in range(CT):
                for th in range(TH):
                    yp = conv_ps.tile([128, 512], F32, tag="conv")
                    r0 = th * 16  # image-row offset of this half
                    for k in range(9):
                        dy, dx = k // 3 - 1, k % 3 - 1
                        off = (r0 + 1 + dy) * 34 + (1 + dx)
                        rhs = bass.AP(
                            tensor=xp[i].tensor,
                            offset=xp[i].offset + off,
                            ap=[list(p) for p in xp[i].ap[:1]] + [[34, 16], [1, 32]],
                        )
                        nc.tensor.matmul(
                            yp[:].rearrange("p (a b) -> p a b", a=16),
                            diag[i][:, k, :],
                            rhs,
                            start=(k == 0),
                            stop=(k == 8),
                        )
                    # + BN bias, round to f32r, store to y
                    nc.vector.tensor_scalar_add(
                        y_sb[i][:, th * 512 : (th + 1) * 512],
                        yp[:],
                        bias_sb[i][:],
                    )
            if debug:
                for i in range(CT):
                    nc.sync.dma_start(
                        dbg["y"][i * 128 : (i + 1) * 128, :], y_sb[i][:].bitcast(F32)
                    )

            # ---- q/k projections: qT[o, t] ----
            # o-tile outer so scores for group g can start once both q and k
            # of that group are done (attention doesn't wait for all of qk)
            for ot in range(CT):
                for nm, dst in (("q", qT_sb), ("k", kT_sb)):
                    for th in range(TH):
                        pp = qk_ps.tile([128, 512], F32, tag="qk")
                        for kt in range(CT):
                            nc.tensor.matmul(
                                pp[:],
                                wT_sb[nm][kt][:, ot * 128 : (ot + 1) * 128],
                                y_sb[kt][:, th * 512 : (th + 1) * 512],
                                start=(kt == 0),
                                stop=(kt == CT - 1),
                            )
                        nc.vector.tensor_copy(
                            dst[ot][:, th * 512 : (th + 1) * 512], pp[:]
                        )
            if debug:
                for i in range(CT):
                    nc.sync.dma_start(
                        dbg["qT"][i * 128 : (i + 1) * 128, :], qT_sb[i][:].bitcast(F32)
                    )

            # ---- v projection: v[t, o], written into vaug [t, h, 34] ----
            for tt in range(TT):
                vp = v_ps.tile([128, C], F32, tag="v")
                for kt in range(CT):
                    nc.tensor.matmul(
                        vp[:],
                        y_sb[kt][:, tt * 128 : (tt + 1) * 128],
                        wT_sb["v"][kt][:],
                        start=(kt == 0),
                        stop=(kt == CT - 1),
                    )
                nc.vector.memset(vaug[tt][:, :, 32:34].bitcast(F32), 0.0)
                nc.vector.memset(vaug[tt][:, :, 32:33].bitcast(F32), 1.0)
                nc.vector.tensor_copy(
                    vaug[tt][:, :, 0:32],
                    vp[:].rearrange("p (h d) -> p h d", h=NH),
                )
            if debug:
                for tt in range(TT):
                    nc.sync.dma_start(
                        dbg["vaug"][tt * 128 : (tt + 1) * 128, :],
                        vaug[tt][:].bitcast(F32).rearrange("p a b -> p (a b)"),
                    )

        # ---- attention ----
        with ExitStack() as ph2:
            s_ps = ph2.enter_context(tc.tile_pool(name="s_ps", bufs=1, space="PSUM"))
            ov_ps = ph2.enter_context(tc.tile_pool(name="ov_ps", bufs=2, space="PSUM"))
            sg_ps = ph2.enter_context(tc.tile_pool(name="sg_ps", bufs=1, space="PSUM"))
            rb_ps = ph2.enter_context(tc.tile_pool(name="rb_ps", bufs=1, space="PSUM"))
            epool = ph2.enter_context(tc.tile_pool(name="epool", bufs=10))
            rpool = ph2.enter_context(tc.tile_pool(name="rpool", bufs=3))
            ovpool = ph2.enter_context(tc.tile_pool(name="ovpool", bufs=6))

            # indicator tiles (host-built): ind4[j][k, m] = 1.0 iff
            # (k, m) == (32, j) — the gather matmul moves a head's sums row
            # (psum partition 32) to partition j of the gather tile.
            # bind[k, m] = 1.0 iff m // 32 == k — the K=4 broadcast matmul
            # bind.T @ rr[0:4] replicates head k's recip row across psum
            # partitions 32k..32k+31 for all 4 heads at once.
            ind4 = []
            for j in range(4):
                it = rpool.tile([34, 128], F32R, tag=f"ind{j}", name=f"ind{j}")
                nc.sync.dma_start(it[:], ind_d[j])
                ind4.append(it)
            bind = rpool.tile([4, 128], F32R, tag="bind", name="bind")
            nc.sync.dma_start(bind[:], bind_d)

            for lh in range(TH):
                for g in range(CT):
                    E = []
                    for tt in range(TT):
                        s4 = s_ps.tile([128, 2048], F32, tag="s4")
                        for hh in range(4):
                            nc.tensor.matmul(
                                s4[:, 512 * hh : 512 * (hh + 1)],
                                kT_sb[g][
                                    32 * hh : 32 * (hh + 1),
                                    tt * 128 : (tt + 1) * 128,
                                ],
                                qT_sb[g][
                                    32 * hh : 32 * (hh + 1),
                                    lh * 512 : (lh + 1) * 512,
                                ],
                                start=True,
                                stop=True,
                                tile_position=(32 * hh, 0),
                            )
                        e = epool.tile([128, 2048], F32R, tag="E")
                        nc.scalar.activation(e[:], s4[:], AF.Exp, scale=SCALE)
                        E.append(e)
                    if debug and g == 0:
                        for tt in range(TT):
                            nc.sync.dma_start(
                                dbg["E0"][lh, tt * 128 : (tt + 1) * 128, :],
                                E[tt][:].bitcast(F32),
                            )
                    if stage < 3:
                        continue

                    ovs_g = []
                    sg = sg_ps.tile([128, 512], F32, tag="sg")
                    for hl in range(4):  # head local to group
                        ov = ov_ps.tile([128, 512], F32, tag="ov")
                        for tt in range(TT):
                            nc.tensor.matmul(
                                ov[0:34, :],
                                vaug[tt][:, 4 * g + hl, :],
                                E[tt][:, 512 * hl : 512 * (hl + 1)],
                                start=(tt == 0),
                                stop=(tt == TT - 1),
                            )
                        ovs = ovpool.tile([128, 512], F32R, tag="ovs")
                        nc.vector.tensor_copy(ovs[0:34, :], ov[0:34, :])
                        ovs_g.append(ovs)
                        if debug and g == 0:
                            nc.sync.dma_start(
                                dbg["ov"][hl, :, lh * 512 : (lh + 1) * 512],
                                ovs[0:34, :].bitcast(F32),
                            )
                        if stage < 4:
                            continue
                        # gather this head's sums row into partition 32*hl of sg
                        nc.tensor.matmul(
                            sg[:],
                            ind4[hl][:],
                            ovs[0:34, :],
                            start=(hl == 0),
                            stop=(hl == 3),
                        )
                    if stage < 4:
                        continue
                    # batched precise reciprocal of the 4 sums rows (other
                    # rows are zeros -> inf, never read)
                    rrf = rpool.tile([128, 512], F32, tag="rrf")
                    nc.vector.reciprocal(rrf[0:4, :], sg[0:4, :])
                    rr = rpool.tile([128, 512], F32R, tag="rr")
                    nc.vector.tensor_copy(rr[0:4, :], rrf[0:4, :])
                    # one K=4 matmul broadcasts all 4 heads' recips to
                    # partitions 32*hl .. 32*hl+31
                    Rb = rb_ps.tile([128, 512], F32, tag="Rb")
                    nc.tensor.matmul(
                        Rb[:], bind[:], rr[0:4, :], start=True, stop=True
                    )
                    for hl in range(4):
                        nc.vector.tensor_tensor(
                            attn_sb[g][
                                32 * hl : 32 * (hl + 1),
                                lh * 512 : (lh + 1) * 512,
                            ],
                            ovs_g[hl][0:32, :].bitcast(F32),
                            Rb[32 * hl : 32 * (hl + 1), :],
                            ALU.mult,
                        )
                        if debug and g == 0:
                            rbc = rpool.tile([128, 512], F32, tag="rbc")
                            nc.vector.tensor_copy(
                                rbc[0:32, :], Rb[32 * hl : 32 * (hl + 1), :]
                            )
                            nc.sync.dma_start(
                                dbg["R"][
                                    32 * hl : 32 * (hl + 1),
                                    lh * 512 : (lh + 1) * 512,
                                ],
                                rbc[0:32, :],
                            )
            if debug and stage >= 4:
                for i in range(CT):
                    nc.sync.dma_start(
                        dbg["attn"][i * 128 : (i + 1) * 128, :],
                        attn_sb[i][:].bitcast(F32),
                    )

        # ---- output projection ----
        with ExitStack() as ph3:
            o_ps = ph3.enter_context(tc.tile_pool(name="o_ps", bufs=3, space="PSUM"))
            for ot in range(CT) if stage >= 5 else []:
                for th in range(TH):
                    op = o_ps.tile([128, 512], F32, tag="o")
                    for kt in range(CT):
                        nc.tensor.matmul(
                            op[:],
                            wT_sb["o"][kt][:, ot * 128 : (ot + 1) * 128],
                            attn_sb[kt][:, th * 512 : (th + 1) * 512],
                            start=(kt == 0),
                            stop=(kt == CT - 1),
                        )
                    oc = copies.tile([128, 512], F32, tag="oc")
                    nc.vector.tensor_copy(oc[:], op[:])
                    nc.sync.dma_start(
                        outT_d[ot * 128 : (ot + 1) * 128, th * 512 : (th + 1) * 512],
                        oc[:],
                    )

    nc.compile()
    return nc


def _prep_inputs(x, conv_w, bn_gamma, bn_beta, bn_mean, bn_var, wq, wk, wv, wo):
    f32 = np.float32
    inv = (bn_gamma / np.sqrt(bn_var + BN_EPS)).astype(f32)
    w9 = (conv_w.reshape(C, 9) * inv[:, None]).astype(f32)
    bias = (bn_beta - bn_mean * inv).astype(f32).reshape(C, 1)
    wqT = np.ascontiguousarray(np.asarray(wq, f32).T)
    wkT = np.ascontiguousarray(np.asarray(wk, f32).T)
    wvT = np.ascontiguousarray(np.asarray(wv, f32).T)
    woT = np.ascontiguousarray(np.asarray(wo, f32).T)
    ind = np.zeros((4, 34, 128), f32)
    for j in range(4):
        ind[j, 32, j] = 1.0
    bind = np.zeros((4, 128), f32)
    for j in range(4):
        bind[j, 32 * j : 32 * (j + 1)] = 1.0
    maps = []
    for b in range(B):
        maps.append(
            {
                "xt": np.ascontiguousarray(np.asarray(x[b], f32).T),
                "w9": w9,
                "bias": bias,
                "wqT": wqT,
                "wkT": wkT,
                "wvT": wvT,
                "woT": woT,
                "ind": ind,
                "bind": bind,
            }
        )
    return maps


def kernel(x, conv_w, bn_gamma, bn_beta, bn_mean, bn_var, wq, wk, wv, wo, h, w,
           **kw):
    assert int(h) == HH and int(w) == WW
    from concourse.bass_utils import run_bass_kernel_spmd

    if "nc" not in _CACHE:
        _CACHE["nc"] = _build()
    nc = _CACHE["nc"]
    maps = _prep_inputs(
        x, conv_w, bn_gamma, bn_beta, bn_mean, bn_var, wq, wk, wv, wo
    )
    res = run_bass_kernel_spmd(nc, maps, list(range(NCORES)))
    out = np.stack([res.results[b]["outT"].T for b in range(B)])
    return out.astype(np.float32)



# revision 33
# speedup vs baseline: 1.2273x; 1.2273x over previous
"""Trainium2 Bass kernel for conv-projected multi-head attention (v5).

Reference computation (per batch item b of 8, one NeuronCore each):
  y   = BN(depthwise3x3(x_b reshaped to [C,32,32]))      # q = k = v = y
  q/k/v = y @ w{q,k,v}^T  (heads: 12 x 32)
  att = softmax((q @ k^T) * sqrt(32))
  out = (att @ v) @ wo^T

v5 vs the fp32r baseline (263 us):
 - conv, q/k/v/out projections and the score matmuls run with bf16
   operands (psum stays f32); sqrt(32) folded into wq host-side.
   NOTE: exp with bf16 output miscompiles (writes raw f32) and walrus
   rejects mixed 32/16-bit matmul inputs, so E and vaug stay f32r and
   PV runs f32r like the baseline.
 - reciprocal_approx_fast replaces the 3.3us-per-call precise reciprocal
 - software-pipelined emission: scores+exp of block k+1 are emitted
   around PV of block k so ACT(exp) always has a backlog; attention for
   group 0 is emitted before the v projection / qk groups 1-2 so the
   scalar engine starts early.
Layout is channel-major: xT [C=384, T=1024] per core; S^T[t, l] per head;
vaug [t, h, 34] with a ones column so PV also yields the softmax
denominators in psum row 32 (gathered via ind4 matmuls, broadcast via a
K=4 bind matmul, exactly as the baseline).
"""
import sys

sys.path.insert(0, "/opt/trn_rl_repo")
from contextlib import ExitStack

import numpy as np

B, T, C = 8, 1024, 384
NH, DH = 12, 32
HH = WW = 32
SCALE = float(DH) ** 0.5
BN_EPS = 1e-5
NCORES = 8

_CACHE = {}


def _build(debug=False):
    import concourse.bass as bass
    import concourse.tile as tile
    from concourse import bacc, mybir
    from concourse.masks import make_identity

    F32 = mybir.dt.float32
    F32R = mybir.dt.float32r
    BF16 = mybir.dt.bfloat16
    AF = mybir.ActivationFunctionType
    ALU = mybir.AluOpType

    nc = bacc.Bacc("TRN2", target_bir_lowering=False, debug=False)

    xt_d = nc.dram_tensor("xt", [C, T], BF16, kind="ExternalInput").ap()
    w9_d = nc.dram_tensor("w9", [C, 9], F32, kind="ExternalInput").ap()
    bias_d = nc.dram_tensor("bias", [C, 1], F32, kind="ExternalInput").ap()
    wqT_d = nc.dram_tensor("wqT", [C, C], BF16, kind="ExternalInput").ap()
    wkT_d = nc.dram_tensor("wkT", [C, C], BF16, kind="ExternalInput").ap()
    wvT_d = nc.dram_tensor("wvT", [C, C], BF16, kind="ExternalInput").ap()
    woT_d = nc.dram_tensor("woT", [C, C], BF16, kind="ExternalInput").ap()
    ind_d = nc.dram_tensor("ind", [4, 34, 128], F32R, kind="ExternalInput").ap()
    bind_d = nc.dram_tensor("bind", [4, 128], F32R, kind="ExternalInput").ap()
    outT_d = nc.dram_tensor("outT", [C, T], F32, kind="ExternalOutput").ap()
    dbg = {}
    if debug:
        dbg["y"] = nc.dram_tensor("dbg_y", [C, T], BF16, kind="ExternalOutput").ap()
        dbg["qT"] = nc.dram_tensor("dbg_qT", [C, T], BF16, kind="ExternalOutput").ap()
        dbg["attn"] = nc.dram_tensor(
            "dbg_attn", [C, T], BF16, kind="ExternalOutput"
        ).ap()
        dbg["E"] = nc.dram_tensor(
            "dbg_E", [8, 128, 2048], F32, kind="ExternalOutput"
        ).ap()

    CT = C // 128  # 3 c-tiles / head groups of 4
    TT = T // 128  # 8 t-tiles
    TH = T // 512  # 2 l-halves
    DBG_BLOCK = (1, 0)  # (lh, g) block to dump in debug mode

    with tile.TileContext(nc) as tc, ExitStack() as top:
        persist = top.enter_context(tc.tile_pool(name="persist", bufs=1))
        copies = top.enter_context(tc.tile_pool(name="copies", bufs=3))

        y_sb = [persist.tile([128, T], BF16, tag=f"y{i}", name=f"y{i}") for i in range(CT)]
        qT_sb = [persist.tile([128, T], BF16, tag=f"q{i}", name=f"q{i}") for i in range(CT)]
        kT_sb = [persist.tile([128, T], BF16, tag=f"k{i}", name=f"k{i}") for i in range(CT)]
        vaug = [persist.tile([128, NH, 34], F32R, tag=f"va{i}", name=f"va{i}") for i in range(TT)]
        attn_sb = [persist.tile([128, T], BF16, tag=f"at{i}", name=f"at{i}") for i in range(CT)]

        # PSUM banks (8 x 2KB), pools strictly LIFO-nested per space:
        #   s_ps 4 (one s4 [128,2048], bufs=1) spans the whole kernel
        #   conv window:  s 4 + conv_ps 2                 = 6
        #   qk/v window:  s 4 + qk_ps 2 + v_ps 2          = 8
        #   attention:    s 4 + ov 2 + sg 1 + rb 1        = 8
        #   out-proj:     s 4 + o_ps 3                    = 7
        s_ps = top.enter_context(tc.tile_pool(name="s_ps", bufs=1, space="PSUM"))

        # ---------------- phase 1: conv ----------------
        ph_conv = ExitStack()
        convpool = ph_conv.enter_context(tc.tile_pool(name="convpool", bufs=1))
        conv_ps = ph_conv.enter_context(tc.tile_pool(name="conv_ps", bufs=2, space="PSUM"))
        ph1 = ExitStack()  # qk_ps/v_ps entered after conv pools close

        xt_sb = [convpool.tile([128, T], BF16, tag=f"xt{i}", name=f"xt{i}") for i in range(CT)]
        xp = [convpool.tile([128, 34 * 34], BF16, tag=f"xp{i}", name=f"xp{i}") for i in range(CT)]
        w9_sb = [convpool.tile([128, 9], F32, tag=f"w9{i}", name=f"w9s{i}") for i in range(CT)]
        ident = convpool.tile([128, 128], F32, tag="ident")
        diag = [convpool.tile([128, 9, 128], BF16, tag=f"dg{i}", name=f"dg{i}") for i in range(CT)]

        make_identity(nc, ident[:])
        for i in range(CT):
            nc.sync.dma_start(xt_sb[i][:], xt_d[i * 128 : (i + 1) * 128, :])
            nc.sync.dma_start(w9_sb[i][:], w9_d[i * 128 : (i + 1) * 128, :])
            nc.vector.memset(xp[i][:], 0.0)
            nc.vector.tensor_copy(
                xp[i][:].rearrange("p (a b) -> p a b", a=34)[:, 1:33, 1:33],
                xt_sb[i][:].rearrange("p (a b) -> p a b", a=32),
            )
            for k in range(9):
                nc.vector.tensor_scalar_mul(
                    diag[i][:, k, :], ident[:], w9_sb[i][:, k : k + 1]
                )

        # weight / constant DMAs after the conv inputs so conv starts early
        wT_sb = {}
        for nm, d in (("q", wqT_d), ("k", wkT_d), ("v", wvT_d), ("o", woT_d)):
            tiles = [persist.tile([128, C], BF16, tag=f"w{nm}{i}", name=f"w{nm}{i}") for i in range(CT)]
            for i in range(CT):
                nc.sync.dma_start(tiles[i][:], d[i * 128 : (i + 1) * 128, :])
            wT_sb[nm] = tiles

        bias_sb = [persist.tile([128, 1], F32, tag=f"b{i}", name=f"b{i}") for i in range(CT)]
        for i in range(CT):
            nc.sync.dma_start(bias_sb[i][:], bias_d[i * 128 : (i + 1) * 128, :])
        ind4 = []
        for j in range(4):
            it = persist.tile([34, 128], F32R, tag=f"ind{j}", name=f"ind{j}")
            nc.sync.dma_start(it[:], ind_d[j])
            ind4.append(it)
        bind = persist.tile([4, 128], F32R, tag="bind", name="bind")
        nc.sync.dma_start(bind[:], bind_d)

        # conv: 9 accumulating diag matmuls per (c-tile, t-half)
        for i in range(CT):
            for th in range(TH):
                yp = conv_ps.tile([128, 512], F32, tag="conv", name=f"yp{i}{th}")
                r0 = th * 16
                for k in range(9):
                    dy, dx = k // 3 - 1, k % 3 - 1
                    off = (r0 + 1 + dy) * 34 + (1 + dx)
                    rhs = bass.AP(
                        tensor=xp[i].tensor,
                        offset=xp[i].offset + off,
                        ap=[list(p) for p in xp[i].ap[:1]] + [[34, 16], [1, 32]],
                    )
                    nc.tensor.matmul(
                        yp[:].rearrange("p (a b) -> p a b", a=16),
                        diag[i][:, k, :],
                        rhs,
                        start=(k == 0),
                        stop=(k == 8),
                    )
                nc.vector.tensor_scalar_add(
                    y_sb[i][:, th * 512 : (th + 1) * 512], yp[:], bias_sb[i][:]
                )
        if debug:
            for i in range(CT):
                nc.sync.dma_start(dbg["y"][i * 128 : (i + 1) * 128, :], y_sb[i][:])

        ps = {}
        pools = {}

        def qk_proj(ot):
            for nm, dst in (("q", qT_sb), ("k", kT_sb)):
                for th in range(TH):
                    pp = ps["qk"].tile([128, 512], F32, tag="qk", name=f"pp{nm}{ot}{th}")
                    for kt in range(CT):
                        nc.tensor.matmul(
                            pp[:],
                            wT_sb[nm][kt][:, ot * 128 : (ot + 1) * 128],
                            y_sb[kt][:, th * 512 : (th + 1) * 512],
                            start=(kt == 0),
                            stop=(kt == CT - 1),
                        )
                    nc.vector.tensor_copy(dst[ot][:, th * 512 : (th + 1) * 512], pp[:])

        def v_proj():
            for tt in range(TT):
                vp = ps["v"].tile([128, C], F32, tag="v", name=f"vp{tt}")
                for kt in range(CT):
                    nc.tensor.matmul(
                        vp[:],
                        y_sb[kt][:, tt * 128 : (tt + 1) * 128],
                        wT_sb["v"][kt][:],
                        start=(kt == 0),
                        stop=(kt == CT - 1),
                    )
                nc.vector.memset(vaug[tt][:, :, 32:34].bitcast(F32), 0.0)
                nc.vector.memset(vaug[tt][:, :, 32:33].bitcast(F32), 1.0)
                nc.vector.tensor_copy(
                    vaug[tt][:, :, 0:32], vp[:].rearrange("p (h d) -> p h d", h=NH)
                )

        def scores_block(lh, g):
            """Scores + exp for (l-half lh, head-group g). Returns E tiles."""
            E = []
            for tt in range(TT):
                s4 = s_ps.tile([128, 2048], F32, tag="s4", name=f"s{lh}{g}{tt}")
                for hh in range(4):
                    nc.tensor.matmul(
                        s4[:, 512 * hh : 512 * (hh + 1)],
                        kT_sb[g][32 * hh : 32 * (hh + 1), tt * 128 : (tt + 1) * 128],
                        qT_sb[g][32 * hh : 32 * (hh + 1), lh * 512 : (lh + 1) * 512],
                        start=True,
                        stop=True,
                        tile_position=(32 * hh, 0),
                    )
                e = pools["e"].tile([128, 2048], F32R, tag="E", name=f"E{lh}{g}{tt}")
                nc.scalar.activation(e[:], s4[:], AF.Exp)
                E.append(e)
            if debug and (lh, g) == DBG_BLOCK:
                for tt in range(TT):
                    nc.sync.dma_start(dbg["E"][tt], E[tt][:].bitcast(F32))
            return E

        def pv_block(lh, g, E):
            """PV + normalize for (lh, g) consuming that block's E tiles."""
            ovs_g = []
            sg = ps["sg"].tile([128, 512], F32, tag="sg", name=f"sg{lh}{g}")
            for hl in range(4):
                ov = ps["ov"].tile([128, 512], F32, tag="ov", name=f"ov{lh}{g}{hl}")
                for tt in range(TT):
                    nc.tensor.matmul(
                        ov[0:34, :],
                        vaug[tt][:, 4 * g + hl, :],
                        E[tt][:, 512 * hl : 512 * (hl + 1)],
                        start=(tt == 0),
                        stop=(tt == TT - 1),
                    )
                ovs = pools["ov"].tile([128, 512], F32R, tag="ovs", name=f"ovs{lh}{g}{hl}")
                nc.vector.tensor_copy(ovs[0:34, :], ov[0:34, :])
                ovs_g.append(ovs)
                # gather this head's sums row into partition 32*hl of sg
                nc.tensor.matmul(
                    sg[:],
                    ind4[hl][:],
                    ovs[0:34, :],
                    start=(hl == 0),
                    stop=(hl == 3),
                )
            rrf = pools["r"].tile([128, 512], F32, tag="rrf", name=f"rrf{lh}{g}")
            nc.vector.reciprocal_approx_fast(rrf[0:4, :], sg[0:4, :])
            rr = pools["r"].tile([128, 512], F32R, tag="rr", name=f"rr{lh}{g}")
            nc.vector.tensor_copy(rr[0:4, :], rrf[0:4, :])
            # one K=4 matmul broadcasts all 4 heads' recips to partitions
            # 32*hl .. 32*hl+31
            Rb = ps["rb"].tile([128, 512], F32, tag="Rb", name=f"Rb{lh}{g}")
            nc.tensor.matmul(
                Rb[:], bind[:], rr[0:4, :], start=True, stop=True
            )
            for hl in range(4):
                nc.vector.tensor_tensor(
                    attn_sb[g][32 * hl : 32 * (hl + 1), lh * 512 : (lh + 1) * 512],
                    ovs_g[hl][0:32, :].bitcast(F32),
                    Rb[32 * hl : 32 * (hl + 1), :],
                    ALU.mult,
                )

        # ---------------- schedule ----------------
        ph_conv.close()
        pools["e"] = top.enter_context(tc.tile_pool(name="epool", bufs=16))
        pools["r"] = top.enter_context(tc.tile_pool(name="rpool", bufs=2))
        pools["ov"] = top.enter_context(tc.tile_pool(name="ovpool", bufs=6))
        ps["qk"] = ph1.enter_context(tc.tile_pool(name="qk_ps", bufs=2, space="PSUM"))
        ps["v"] = ph1.enter_context(tc.tile_pool(name="v_ps", bufs=2, space="PSUM"))
        qk_proj(0)
        E00 = scores_block(0, 0)
        E10 = scores_block(1, 0)
        v_proj()
        qk_proj(1)
        qk_proj(2)
        ph1.close()
        ph2 = ExitStack()
        ps["ov"] = ph2.enter_context(tc.tile_pool(name="ov_ps", bufs=2, space="PSUM"))
        ps["sg"] = ph2.enter_context(tc.tile_pool(name="sg_ps", bufs=1, space="PSUM"))
        ps["rb"] = ph2.enter_context(tc.tile_pool(name="rb_ps", bufs=1, space="PSUM"))

        pv_block(0, 0, E00)
        E01 = scores_block(0, 1)
        pv_block(1, 0, E10)
        E11 = scores_block(1, 1)
        pv_block(0, 1, E01)
        E02 = scores_block(0, 2)
        pv_block(1, 1, E11)
        E12 = scores_block(1, 2)
        pv_block(0, 2, E02)
        pv_block(1, 2, E12)
        ph2.close()
        if debug:
            for i in range(CT):
                nc.sync.dma_start(dbg["qT"][i * 128 : (i + 1) * 128, :], qT_sb[i][:])
                nc.sync.dma_start(dbg["attn"][i * 128 : (i + 1) * 128, :], attn_sb[i][:])

        # ---------------- output projection ----------------
        with tc.tile_pool(name="o_ps", bufs=3, space="PSUM") as o_ps:
            for ot in range(CT):
                for th in range(TH):
                    op = o_ps.tile([128, 512], F32, tag="o", name=f"op{ot}{th}")
                    for kt in range(CT):
                        nc.tensor.matmul(
                            op[:],
                            wT_sb["o"][kt][:, ot * 128 : (ot + 1) * 128],
                            attn_sb[kt][:, th * 512 : (th + 1) * 512],
                            start=(kt == 0),
                            stop=(kt == CT - 1),
                        )
                    oc = copies.tile([128, 512], F32, tag="oc", name=f"oc{ot}{th}")
                    nc.vector.tensor_copy(oc[:], op[:])
                    nc.sync.dma_start(
                        outT_d[ot * 128 : (ot + 1) * 128, th * 512 : (th + 1) * 512],
                        oc[:],
                    )

    nc.compile()
    return nc


def _prep_inputs(x, conv_w, bn_gamma, bn_beta, bn_mean, bn_var, wq, wk, wv, wo):
    import ml_dtypes

    f32 = np.float32
    bf16 = ml_dtypes.bfloat16
    inv = (bn_gamma / np.sqrt(bn_var + BN_EPS)).astype(f32)
    w9 = (conv_w.reshape(C, 9) * inv[:, None]).astype(f32)
    bias = (bn_beta - bn_mean * inv).astype(f32).reshape(C, 1)
    wqT = np.ascontiguousarray((np.asarray(wq, f32) * f32(SCALE)).T).astype(bf16)
    wkT = np.ascontiguousarray(np.asarray(wk, f32).T).astype(bf16)
    wvT = np.ascontiguousarray(np.asarray(wv, f32).T).astype(bf16)
    woT = np.ascontiguousarray(np.asarray(wo, f32).T).astype(bf16)
    ind = np.zeros((4, 34, 128), f32)
    for j in range(4):
        ind[j, 32, j] = 1.0
    bind = np.zeros((4, 128), f32)
    for j in range(4):
        bind[j, 32 * j : 32 * (j + 1)] = 1.0
    maps = []
    for b in range(B):
        maps.append(
            {
                "xt": np.ascontiguousarray(np.asarray(x[b], f32).T).astype(bf16),
                "w9": w9,
                "bias": bias,
                "wqT": wqT,
                "wkT": wkT,
                "wvT": wvT,
                "woT": woT,
                "ind": ind,
                "bind": bind,
            }
        )
    return maps


def kernel(x, conv_w, bn_gamma, bn_beta, bn_mean, bn_var, wq, wk, wv, wo, h, w,
           **kw):
    assert int(h) == HH and int(w) == WW
    from concourse.bass_utils import run_bass_kernel_spmd

    if "nc" not in _CACHE:
        _CACHE["nc"] = _build()
    nc = _CACHE["nc"]
    maps = _prep_inputs(
        x, conv_w, bn_gamma, bn_beta, bn_mean, bn_var, wq, wk, wv, wo
    )
    res = run_bass_kernel_spmd(nc, maps, list(range(NCORES)))
    out = np.stack([res.results[b]["outT"].T for b in range(B)])
    return out.astype(np.float32)


# revision 34
# speedup vs baseline: 1.3143x; 1.0709x over previous
"""Trainium2 Bass kernel for conv-projected multi-head attention (v5).

Reference computation (per batch item b of 8, one NeuronCore each):
  y   = BN(depthwise3x3(x_b reshaped to [C,32,32]))      # q = k = v = y
  q/k/v = y @ w{q,k,v}^T  (heads: 12 x 32)
  att = softmax((q @ k^T) * sqrt(32))
  out = (att @ v) @ wo^T

v5 vs the fp32r baseline (263 us):
 - conv, q/k/v/out projections and the score matmuls run with bf16
   operands (psum stays f32); sqrt(32) folded into wq host-side.
   NOTE: exp with bf16 output miscompiles (writes raw f32) and walrus
   rejects mixed 32/16-bit matmul inputs, so E and vaug stay f32r and
   PV runs f32r like the baseline.
 - reciprocal_approx_fast replaces the 3.3us-per-call precise reciprocal
 - software-pipelined emission: scores+exp of block k+1 are emitted
   around PV of block k so ACT(exp) always has a backlog; attention for
   group 0 is emitted before the v projection / qk groups 1-2 so the
   scalar engine starts early.
Layout is channel-major: xT [C=384, T=1024] per core; S^T[t, l] per head;
vaug [t, h, 34] with a ones column so PV also yields the softmax
denominators in psum row 32 (gathered via ind4 matmuls, broadcast via a
K=4 bind matmul, exactly as the baseline).
"""
import sys

sys.path.insert(0, "/opt/trn_rl_repo")
from contextlib import ExitStack

import numpy as np

B, T, C = 8, 1024, 384
NH, DH = 12, 32
HH = WW = 32
SCALE = float(DH) ** 0.5
BN_EPS = 1e-5
NCORES = 8

_CACHE = {}


def _build(debug=False):
    import concourse.bass as bass
    import concourse.tile as tile
    from concourse import bacc, mybir
    from concourse.masks import make_identity

    F32 = mybir.dt.float32
    F32R = mybir.dt.float32r
    BF16 = mybir.dt.bfloat16
    AF = mybir.ActivationFunctionType
    ALU = mybir.AluOpType

    nc = bacc.Bacc("TRN2", target_bir_lowering=False, debug=False)

    xt_d = nc.dram_tensor("xt", [C, T], BF16, kind="ExternalInput").ap()
    w9_d = nc.dram_tensor("w9", [C, 9], F32, kind="ExternalInput").ap()
    bias_d = nc.dram_tensor("bias", [C, 1], F32, kind="ExternalInput").ap()
    wqT_d = nc.dram_tensor("wqT", [C, C], BF16, kind="ExternalInput").ap()
    wkT_d = nc.dram_tensor("wkT", [C, C], BF16, kind="ExternalInput").ap()
    wvT_d = nc.dram_tensor("wvT", [C, C], BF16, kind="ExternalInput").ap()
    woT_d = nc.dram_tensor("woT", [C, C], BF16, kind="ExternalInput").ap()
    ind_d = nc.dram_tensor("ind", [4, 34, 128], F32R, kind="ExternalInput").ap()
    bind_d = nc.dram_tensor("bind", [4, 128], F32R, kind="ExternalInput").ap()
    outT_d = nc.dram_tensor("outT", [C, T], F32, kind="ExternalOutput").ap()
    dbg = {}
    if debug:
        dbg["y"] = nc.dram_tensor("dbg_y", [C, T], BF16, kind="ExternalOutput").ap()
        dbg["qT"] = nc.dram_tensor("dbg_qT", [C, T], BF16, kind="ExternalOutput").ap()
        dbg["attn"] = nc.dram_tensor(
            "dbg_attn", [C, T], BF16, kind="ExternalOutput"
        ).ap()
        dbg["E"] = nc.dram_tensor(
            "dbg_E", [8, 128, 2048], F32, kind="ExternalOutput"
        ).ap()

    CT = C // 128  # 3 c-tiles / head groups of 4
    TT = T // 128  # 8 t-tiles
    TH = T // 512  # 2 l-halves
    DBG_BLOCK = (1, 0)  # (lh, g) block to dump in debug mode

    with tile.TileContext(nc) as tc, ExitStack() as top:
        persist = top.enter_context(tc.tile_pool(name="persist", bufs=1))
        copies = top.enter_context(tc.tile_pool(name="copies", bufs=3))

        y_sb = [persist.tile([128, T], BF16, tag=f"y{i}", name=f"y{i}") for i in range(CT)]
        qT_sb = [persist.tile([128, T], BF16, tag=f"q{i}", name=f"q{i}") for i in range(CT)]
        kT_sb = [persist.tile([128, T], BF16, tag=f"k{i}", name=f"k{i}") for i in range(CT)]
        vaug = [persist.tile([128, NH, 34], F32R, tag=f"va{i}", name=f"va{i}") for i in range(TT)]
        attn_sb = [persist.tile([128, T], BF16, tag=f"at{i}", name=f"at{i}") for i in range(CT)]

        # PSUM banks (8 x 2KB), pools strictly LIFO-nested per space:
        #   s_ps 4 (one s4 [128,2048], bufs=1) spans the whole kernel
        #   conv window:  s 4 + conv_ps 2                 = 6
        #   qk/v window:  s 4 + qk_ps 2 + v_ps 2          = 8
        #   attention:    s 4 + ov 2 + sg 1 + rb 1        = 8
        #   out-proj:     s 4 + o_ps 3                    = 7
        s_ps = top.enter_context(tc.tile_pool(name="s_ps", bufs=2, space="PSUM"))

        # ---------------- phase 1: conv ----------------
        ph_conv = ExitStack()
        convpool = ph_conv.enter_context(tc.tile_pool(name="convpool", bufs=1))
        conv_ps = ph_conv.enter_context(tc.tile_pool(name="conv_ps", bufs=2, space="PSUM"))
        ph1 = ExitStack()  # qk_ps/v_ps entered after conv pools close

        xt_sb = [convpool.tile([128, T], BF16, tag=f"xt{i}", name=f"xt{i}") for i in range(CT)]
        xp = [convpool.tile([128, 34 * 34], BF16, tag=f"xp{i}", name=f"xp{i}") for i in range(CT)]
        w9_sb = [convpool.tile([128, 9], F32, tag=f"w9{i}", name=f"w9s{i}") for i in range(CT)]
        ident = convpool.tile([128, 128], F32, tag="ident")
        diag = [convpool.tile([128, 9, 128], BF16, tag=f"dg{i}", name=f"dg{i}") for i in range(CT)]

        make_identity(nc, ident[:])
        for tt in range(TT):
            nc.gpsimd.memset(vaug[tt][:, :, 32:34].bitcast(F32), 0.0)
            nc.gpsimd.memset(vaug[tt][:, :, 32:33].bitcast(F32), 1.0)
        for i in range(CT):
            nc.sync.dma_start(xt_sb[i][:], xt_d[i * 128 : (i + 1) * 128, :])
            nc.sync.dma_start(w9_sb[i][:], w9_d[i * 128 : (i + 1) * 128, :])
            nc.vector.memset(xp[i][:], 0.0)
            nc.vector.tensor_copy(
                xp[i][:].rearrange("p (a b) -> p a b", a=34)[:, 1:33, 1:33],
                xt_sb[i][:].rearrange("p (a b) -> p a b", a=32),
            )
            for k in range(9):
                nc.gpsimd.tensor_scalar_mul(
                    diag[i][:, k, :], ident[:], w9_sb[i][:, k : k + 1]
                )

        # weight / constant DMAs after the conv inputs so conv starts early
        wT_sb = {}
        for nm, d in (("q", wqT_d), ("k", wkT_d), ("v", wvT_d), ("o", woT_d)):
            tiles = [persist.tile([128, C], BF16, tag=f"w{nm}{i}", name=f"w{nm}{i}") for i in range(CT)]
            for i in range(CT):
                nc.sync.dma_start(tiles[i][:], d[i * 128 : (i + 1) * 128, :])
            wT_sb[nm] = tiles

        bias_sb = [persist.tile([128, 1], F32, tag=f"b{i}", name=f"b{i}") for i in range(CT)]
        for i in range(CT):
            nc.sync.dma_start(bias_sb[i][:], bias_d[i * 128 : (i + 1) * 128, :])
        ind4 = []
        for j in range(4):
            it = persist.tile([34, 128], F32R, tag=f"ind{j}", name=f"ind{j}")
            nc.sync.dma_start(it[:], ind_d[j])
            ind4.append(it)
        bind = persist.tile([4, 128], F32R, tag="bind", name="bind")
        nc.sync.dma_start(bind[:], bind_d)

        # conv: 9 accumulating diag matmuls per (c-tile, t-half)
        for i in range(CT):
            for th in range(TH):
                yp = conv_ps.tile([128, 512], F32, tag="conv", name=f"yp{i}{th}")
                r0 = th * 16
                for k in range(9):
                    dy, dx = k // 3 - 1, k % 3 - 1
                    off = (r0 + 1 + dy) * 34 + (1 + dx)
                    rhs = bass.AP(
                        tensor=xp[i].tensor,
                        offset=xp[i].offset + off,
                        ap=[list(p) for p in xp[i].ap[:1]] + [[34, 16], [1, 32]],
                    )
                    nc.tensor.matmul(
                        yp[:].rearrange("p (a b) -> p a b", a=16),
                        diag[i][:, k, :],
                        rhs,
                        start=(k == 0),
                        stop=(k == 8),
                    )
                nc.vector.tensor_scalar_add(
                    y_sb[i][:, th * 512 : (th + 1) * 512], yp[:], bias_sb[i][:]
                )
        if debug:
            for i in range(CT):
                nc.sync.dma_start(dbg["y"][i * 128 : (i + 1) * 128, :], y_sb[i][:])

        ps = {}
        pools = {}

        def qk_proj(ot):
            for nm, dst in (("q", qT_sb), ("k", kT_sb)):
                for th in range(TH):
                    pp = ps["qk"].tile([128, 512], F32, tag="qk", name=f"pp{nm}{ot}{th}")
                    for kt in range(CT):
                        nc.tensor.matmul(
                            pp[:],
                            wT_sb[nm][kt][:, ot * 128 : (ot + 1) * 128],
                            y_sb[kt][:, th * 512 : (th + 1) * 512],
                            start=(kt == 0),
                            stop=(kt == CT - 1),
                        )
                    nc.vector.tensor_copy(dst[ot][:, th * 512 : (th + 1) * 512], pp[:])

        def v_proj():
            for tt in range(TT):
                vp = ps["v"].tile([128, C], F32, tag="v", name=f"vp{tt}")
                for kt in range(CT):
                    nc.tensor.matmul(
                        vp[:],
                        y_sb[kt][:, tt * 128 : (tt + 1) * 128],
                        wT_sb["v"][kt][:],
                        start=(kt == 0),
                        stop=(kt == CT - 1),
                    )
                nc.vector.tensor_copy(
                    vaug[tt][:, :, 0:32], vp[:].rearrange("p (h d) -> p h d", h=NH)
                )

        def scores_block(lh, g):
            """Scores + exp for (l-half lh, head-group g). Returns E[p][tt]
            tiles, each [128, 1024] covering head pair (2p, 2p+1)."""
            E = [[None] * TT for _ in range(2)]
            for p in range(2):
                for tt in range(TT):
                    s2 = s_ps.tile([128, 1024], F32, tag="s2", name=f"s{lh}{g}{p}{tt}")
                    for hx in range(2):
                        hh = 2 * p + hx
                        nc.tensor.matmul(
                            s2[:, 512 * hx : 512 * (hx + 1)],
                            kT_sb[g][32 * hh : 32 * (hh + 1), tt * 128 : (tt + 1) * 128],
                            qT_sb[g][32 * hh : 32 * (hh + 1), lh * 512 : (lh + 1) * 512],
                            start=True,
                            stop=True,
                            tile_position=(32 * hh, 0),
                        )
                    e = pools["e"].tile([128, 1024], F32R, tag="E", name=f"E{lh}{g}{p}{tt}")
                    nc.scalar.activation(e[:], s2[:], AF.Exp)
                    E[p][tt] = e
            if debug and (lh, g) == DBG_BLOCK:
                for tt in range(TT):
                    nc.sync.dma_start(dbg["E"][tt][:, 0:1024], E[0][tt][:].bitcast(F32))
                    nc.sync.dma_start(dbg["E"][tt][:, 1024:2048], E[1][tt][:].bitcast(F32))
            return E

        def pv_block(lh, g, E):
            """PV + normalize for (lh, g) consuming that block's E tiles."""
            ovs_g = []
            sg = ps["sg"].tile([128, 512], F32, tag="sg", name=f"sg{lh}{g}")
            for hl in range(4):
                ov = ps["ov"].tile([128, 512], F32, tag="ov", name=f"ov{lh}{g}{hl}")
                for tt in range(TT):
                    nc.tensor.matmul(
                        ov[0:34, :],
                        vaug[tt][:, 4 * g + hl, :],
                        E[hl // 2][tt][:, 512 * (hl % 2) : 512 * (hl % 2 + 1)],
                        start=(tt == 0),
                        stop=(tt == TT - 1),
                    )
                ovs = pools["ov"].tile([128, 512], F32R, tag="ovs", name=f"ovs{lh}{g}{hl}")
                nc.vector.tensor_copy(ovs[0:34, :], ov[0:34, :])
                ovs_g.append(ovs)
                # gather this head's sums row into partition 32*hl of sg
                nc.tensor.matmul(
                    sg[:],
                    ind4[hl][:],
                    ovs[0:34, :],
                    start=(hl == 0),
                    stop=(hl == 3),
                )
            rrf = pools["r"].tile([128, 512], F32, tag="rrf", name=f"rrf{lh}{g}")
            nc.vector.reciprocal_approx_fast(rrf[0:4, :], sg[0:4, :])
            rr = pools["r"].tile([128, 512], F32R, tag="rr", name=f"rr{lh}{g}")
            nc.vector.tensor_copy(rr[0:4, :], rrf[0:4, :])
            # one K=4 matmul broadcasts all 4 heads' recips to partitions
            # 32*hl .. 32*hl+31
            Rb = ps["rb"].tile([128, 512], F32, tag="Rb", name=f"Rb{lh}{g}")
            nc.tensor.matmul(
                Rb[:], bind[:], rr[0:4, :], start=True, stop=True
            )
            for hl in range(4):
                nc.vector.tensor_tensor(
                    attn_sb[g][32 * hl : 32 * (hl + 1), lh * 512 : (lh + 1) * 512],
                    ovs_g[hl][0:32, :].bitcast(F32),
                    Rb[32 * hl : 32 * (hl + 1), :],
                    ALU.mult,
                )

        # ---------------- schedule ----------------
        ph_conv.close()
        pools["e"] = top.enter_context(tc.tile_pool(name="epool", bufs=32))
        pools["r"] = top.enter_context(tc.tile_pool(name="rpool", bufs=2))
        pools["ov"] = top.enter_context(tc.tile_pool(name="ovpool", bufs=6))
        ps["qk"] = ph1.enter_context(tc.tile_pool(name="qk_ps", bufs=2, space="PSUM"))
        ps["v"] = ph1.enter_context(tc.tile_pool(name="v_ps", bufs=2, space="PSUM"))
        qk_proj(0)
        E00 = scores_block(0, 0)
        E10 = scores_block(1, 0)
        v_proj()
        qk_proj(1)
        qk_proj(2)
        ph1.close()
        ph2 = ExitStack()
        ps["ov"] = ph2.enter_context(tc.tile_pool(name="ov_ps", bufs=2, space="PSUM"))
        ps["sg"] = ph2.enter_context(tc.tile_pool(name="sg_ps", bufs=1, space="PSUM"))
        ps["rb"] = ph2.enter_context(tc.tile_pool(name="rb_ps", bufs=1, space="PSUM"))

        pv_block(0, 0, E00)
        E01 = scores_block(0, 1)
        pv_block(1, 0, E10)
        E11 = scores_block(1, 1)
        pv_block(0, 1, E01)
        E02 = scores_block(0, 2)
        pv_block(1, 1, E11)
        E12 = scores_block(1, 2)
        pv_block(0, 2, E02)
        pv_block(1, 2, E12)
        ph2.close()
        if debug:
            for i in range(CT):
                nc.sync.dma_start(dbg["qT"][i * 128 : (i + 1) * 128, :], qT_sb[i][:])
                nc.sync.dma_start(dbg["attn"][i * 128 : (i + 1) * 128, :], attn_sb[i][:])

        # ---------------- output projection ----------------
        with tc.tile_pool(name="o_ps", bufs=3, space="PSUM") as o_ps:
            for ot in range(CT):
                for th in range(TH):
                    op = o_ps.tile([128, 512], F32, tag="o", name=f"op{ot}{th}")
                    for kt in range(CT):
                        nc.tensor.matmul(
                            op[:],
                            wT_sb["o"][kt][:, ot * 128 : (ot + 1) * 128],
                            attn_sb[kt][:, th * 512 : (th + 1) * 512],
                            start=(kt == 0),
                            stop=(kt == CT - 1),
                        )
                    oc = copies.tile([128, 512], F32, tag="oc", name=f"oc{ot}{th}")
                    nc.vector.tensor_copy(oc[:], op[:])
                    nc.sync.dma_start(
                        outT_d[ot * 128 : (ot + 1) * 128, th * 512 : (th + 1) * 512],
                        oc[:],
                    )

    nc.compile()
    return nc


def _prep_inputs(x, conv_w, bn_gamma, bn_beta, bn_mean, bn_var, wq, wk, wv, wo):
    import ml_dtypes

    f32 = np.float32
    bf16 = ml_dtypes.bfloat16
    inv = (bn_gamma / np.sqrt(bn_var + BN_EPS)).astype(f32)
    w9 = (conv_w.reshape(C, 9) * inv[:, None]).astype(f32)
    bias = (bn_beta - bn_mean * inv).astype(f32).reshape(C, 1)
    wqT = np.ascontiguousarray((np.asarray(wq, f32) * f32(SCALE)).T).astype(bf16)
    wkT = np.ascontiguousarray(np.asarray(wk, f32).T).astype(bf16)
    wvT = np.ascontiguousarray(np.asarray(wv, f32).T).astype(bf16)
    woT = np.ascontiguousarray(np.asarray(wo, f32).T).astype(bf16)
    ind = np.zeros((4, 34, 128), f32)
    for j in range(4):
        ind[j, 32, j] = 1.0
    bind = np.zeros((4, 128), f32)
    for j in range(4):
        bind[j, 32 * j : 32 * (j + 1)] = 1.0
    maps = []
    for b in range(B):
        maps.append(
            {
                "xt": np.ascontiguousarray(np.asarray(x[b], f32).T).astype(bf16),
                "w9": w9,
                "bias": bias,
                "wqT": wqT,
                "wkT": wkT,
                "wvT": wvT,
                "woT": woT,
                "ind": ind,
                "bind": bind,
            }
        )
    return maps


def kernel(x, conv_w, bn_gamma, bn_beta, bn_mean, bn_var, wq, wk, wv, wo, h, w,
           **kw):
    assert int(h) == HH and int(w) == WW
    from concourse.bass_utils import run_bass_kernel_spmd

    if "nc" not in _CACHE:
        _CACHE["nc"] = _build()
    nc = _CACHE["nc"]
    maps = _prep_inputs(
        x, conv_w, bn_gamma, bn_beta, bn_mean, bn_var, wq, wk, wv, wo
    )
    res = run_bass_kernel_spmd(nc, maps, list(range(NCORES)))
    out = np.stack([res.results[b]["outT"].T for b in range(B)])
    return out.astype(np.float32)


# revision 35
# speedup vs baseline: 1.3378x; 1.0178x over previous
"""Trainium2 Bass kernel for conv-projected multi-head attention (v5).

Reference computation (per batch item b of 8, one NeuronCore each):
  y   = BN(depthwise3x3(x_b reshaped to [C,32,32]))      # q = k = v = y
  q/k/v = y @ w{q,k,v}^T  (heads: 12 x 32)
  att = softmax((q @ k^T) * sqrt(32))
  out = (att @ v) @ wo^T

v5 vs the fp32r baseline (263 us):
 - conv, q/k/v/out projections and the score matmuls run with bf16
   operands (psum stays f32); sqrt(32) folded into wq host-side.
   NOTE: exp with bf16 output miscompiles (writes raw f32) and walrus
   rejects mixed 32/16-bit matmul inputs, so E and vaug stay f32r and
   PV runs f32r like the baseline.
 - reciprocal_approx_fast replaces the 3.3us-per-call precise reciprocal
 - software-pipelined emission: scores+exp of block k+1 are emitted
   around PV of block k so ACT(exp) always has a backlog; attention for
   group 0 is emitted before the v projection / qk groups 1-2 so the
   scalar engine starts early.
Layout is channel-major: xT [C=384, T=1024] per core; S^T[t, l] per head;
vaug [t, h, 34] with a ones column so PV also yields the softmax
denominators in psum row 32 (gathered via ind4 matmuls, broadcast via a
K=4 bind matmul, exactly as the baseline).
"""
import sys

sys.path.insert(0, "/opt/trn_rl_repo")
from contextlib import ExitStack

import numpy as np

B, T, C = 8, 1024, 384
NH, DH = 12, 32
HH = WW = 32
SCALE = float(DH) ** 0.5
BN_EPS = 1e-5
NCORES = 8

_CACHE = {}


def _build(debug=False):
    import concourse.bass as bass
    import concourse.tile as tile
    from concourse import bacc, mybir
    from concourse.masks import make_identity

    F32 = mybir.dt.float32
    F32R = mybir.dt.float32r
    BF16 = mybir.dt.bfloat16
    AF = mybir.ActivationFunctionType
    ALU = mybir.AluOpType

    nc = bacc.Bacc("TRN2", target_bir_lowering=False, debug=False)

    xt_d = nc.dram_tensor("xt", [C, T], BF16, kind="ExternalInput").ap()
    w9_d = nc.dram_tensor("w9", [C, 9], F32, kind="ExternalInput").ap()
    bias_d = nc.dram_tensor("bias", [C, 1], F32, kind="ExternalInput").ap()
    wqT_d = nc.dram_tensor("wqT", [C, C], BF16, kind="ExternalInput").ap()
    wkT_d = nc.dram_tensor("wkT", [C, C], BF16, kind="ExternalInput").ap()
    wvT_d = nc.dram_tensor("wvT", [C, C], BF16, kind="ExternalInput").ap()
    woT_d = nc.dram_tensor("woT", [C, C], BF16, kind="ExternalInput").ap()
    ind_d = nc.dram_tensor("ind", [4, 34, 128], F32R, kind="ExternalInput").ap()
    bind_d = nc.dram_tensor("bind", [4, 128], F32R, kind="ExternalInput").ap()
    outT_d = nc.dram_tensor("outT", [C, T], F32, kind="ExternalOutput").ap()
    dbg = {}
    if debug:
        dbg["y"] = nc.dram_tensor("dbg_y", [C, T], BF16, kind="ExternalOutput").ap()
        dbg["qT"] = nc.dram_tensor("dbg_qT", [C, T], BF16, kind="ExternalOutput").ap()
        dbg["attn"] = nc.dram_tensor(
            "dbg_attn", [C, T], BF16, kind="ExternalOutput"
        ).ap()
        dbg["E"] = nc.dram_tensor(
            "dbg_E", [8, 128, 2048], BF16, kind="ExternalOutput"
        ).ap()

    CT = C // 128  # 3 c-tiles / head groups of 4
    TT = T // 128  # 8 t-tiles
    TH = T // 512  # 2 l-halves
    DBG_BLOCK = (1, 0)  # (lh, g) block to dump in debug mode

    with tile.TileContext(nc) as tc, ExitStack() as top:
        persist = top.enter_context(tc.tile_pool(name="persist", bufs=1))
        copies = top.enter_context(tc.tile_pool(name="copies", bufs=3))

        y_sb = [persist.tile([128, T], BF16, tag=f"y{i}", name=f"y{i}") for i in range(CT)]
        qT_sb = [persist.tile([128, T], BF16, tag=f"q{i}", name=f"q{i}") for i in range(CT)]
        kT_sb = [persist.tile([128, T], BF16, tag=f"k{i}", name=f"k{i}") for i in range(CT)]
        vaug = [persist.tile([128, NH, 34], BF16, tag=f"va{i}", name=f"va{i}") for i in range(TT)]
        attn_sb = [persist.tile([128, T], BF16, tag=f"at{i}", name=f"at{i}") for i in range(CT)]

        # PSUM banks (8 x 2KB), pools strictly LIFO-nested per space:
        #   s_ps 4 (one s4 [128,2048], bufs=1) spans the whole kernel
        #   conv window:  s 4 + conv_ps 2                 = 6
        #   qk/v window:  s 4 + qk_ps 2 + v_ps 2          = 8
        #   attention:    s 4 + ov 2 + sg 1 + rb 1        = 8
        #   out-proj:     s 4 + o_ps 3                    = 7
        s_ps = top.enter_context(tc.tile_pool(name="s_ps", bufs=2, space="PSUM"))

        # ---------------- phase 1: conv ----------------
        ph_conv = ExitStack()
        convpool = ph_conv.enter_context(tc.tile_pool(name="convpool", bufs=1))
        conv_ps = ph_conv.enter_context(tc.tile_pool(name="conv_ps", bufs=2, space="PSUM"))
        ph1 = ExitStack()  # qk_ps/v_ps entered after conv pools close

        xt_sb = [convpool.tile([128, T], BF16, tag=f"xt{i}", name=f"xt{i}") for i in range(CT)]
        xp = [convpool.tile([128, 34 * 34], BF16, tag=f"xp{i}", name=f"xp{i}") for i in range(CT)]
        w9_sb = [convpool.tile([128, 9], F32, tag=f"w9{i}", name=f"w9s{i}") for i in range(CT)]
        ident = convpool.tile([128, 128], F32, tag="ident")
        diag = [convpool.tile([128, 9, 128], BF16, tag=f"dg{i}", name=f"dg{i}") for i in range(CT)]

        make_identity(nc, ident[:])
        for tt in range(TT):
            nc.gpsimd.memset(vaug[tt][:, :, 32:34], 0.0)
            nc.gpsimd.memset(vaug[tt][:, :, 32:33], 1.0)
        for i in range(CT):
            nc.sync.dma_start(xt_sb[i][:], xt_d[i * 128 : (i + 1) * 128, :])
            nc.sync.dma_start(w9_sb[i][:], w9_d[i * 128 : (i + 1) * 128, :])
            nc.vector.memset(xp[i][:], 0.0)
            nc.vector.tensor_copy(
                xp[i][:].rearrange("p (a b) -> p a b", a=34)[:, 1:33, 1:33],
                xt_sb[i][:].rearrange("p (a b) -> p a b", a=32),
            )
            for k in range(9):
                nc.gpsimd.tensor_scalar_mul(
                    diag[i][:, k, :], ident[:], w9_sb[i][:, k : k + 1]
                )

        # weight / constant DMAs after the conv inputs so conv starts early
        wT_sb = {}
        for nm, d in (("q", wqT_d), ("k", wkT_d), ("v", wvT_d), ("o", woT_d)):
            tiles = [persist.tile([128, C], BF16, tag=f"w{nm}{i}", name=f"w{nm}{i}") for i in range(CT)]
            for i in range(CT):
                nc.sync.dma_start(tiles[i][:], d[i * 128 : (i + 1) * 128, :])
            wT_sb[nm] = tiles

        bias_sb = [persist.tile([128, 1], F32, tag=f"b{i}", name=f"b{i}") for i in range(CT)]
        for i in range(CT):
            nc.sync.dma_start(bias_sb[i][:], bias_d[i * 128 : (i + 1) * 128, :])
        ind4 = []
        for j in range(4):
            it = persist.tile([34, 128], F32R, tag=f"ind{j}", name=f"ind{j}")
            nc.sync.dma_start(it[:], ind_d[j])
            ind4.append(it)
        bind = persist.tile([4, 128], F32R, tag="bind", name="bind")
        nc.sync.dma_start(bind[:], bind_d)

        # conv: 9 accumulating diag matmuls per (c-tile, t-half)
        for i in range(CT):
            for th in range(TH):
                yp = conv_ps.tile([128, 512], F32, tag="conv", name=f"yp{i}{th}")
                r0 = th * 16
                for k in range(9):
                    dy, dx = k // 3 - 1, k % 3 - 1
                    off = (r0 + 1 + dy) * 34 + (1 + dx)
                    rhs = bass.AP(
                        tensor=xp[i].tensor,
                        offset=xp[i].offset + off,
                        ap=[list(p) for p in xp[i].ap[:1]] + [[34, 16], [1, 32]],
                    )
                    nc.tensor.matmul(
                        yp[:].rearrange("p (a b) -> p a b", a=16),
                        diag[i][:, k, :],
                        rhs,
                        start=(k == 0),
                        stop=(k == 8),
                    )
                nc.vector.tensor_scalar_add(
                    y_sb[i][:, th * 512 : (th + 1) * 512], yp[:], bias_sb[i][:]
                )
        if debug:
            for i in range(CT):
                nc.sync.dma_start(dbg["y"][i * 128 : (i + 1) * 128, :], y_sb[i][:])

        ps = {}
        pools = {}

        def qk_proj(ot):
            for nm, dst in (("q", qT_sb), ("k", kT_sb)):
                for th in range(TH):
                    pp = ps["qk"].tile([128, 512], F32, tag="qk", name=f"pp{nm}{ot}{th}")
                    for kt in range(CT):
                        nc.tensor.matmul(
                            pp[:],
                            wT_sb[nm][kt][:, ot * 128 : (ot + 1) * 128],
                            y_sb[kt][:, th * 512 : (th + 1) * 512],
                            start=(kt == 0),
                            stop=(kt == CT - 1),
                        )
                    nc.vector.tensor_copy(dst[ot][:, th * 512 : (th + 1) * 512], pp[:])

        def v_proj():
            for tt in range(TT):
                vp = ps["v"].tile([128, C], F32, tag="v", name=f"vp{tt}")
                for kt in range(CT):
                    nc.tensor.matmul(
                        vp[:],
                        y_sb[kt][:, tt * 128 : (tt + 1) * 128],
                        wT_sb["v"][kt][:],
                        start=(kt == 0),
                        stop=(kt == CT - 1),
                    )
                nc.vector.tensor_copy(
                    vaug[tt][:, :, 0:32], vp[:].rearrange("p (h d) -> p h d", h=NH)
                )

        def scores_block(lh, g):
            """Scores + exp for (l-half lh, head-group g). Returns E[p][tt]
            tiles, each [128, 1024] covering head pair (2p, 2p+1)."""
            E = [[None] * TT for _ in range(2)]
            for p in range(2):
                for tt in range(TT):
                    s2 = s_ps.tile([128, 1024], F32, tag="s2", name=f"s{lh}{g}{p}{tt}")
                    for hx in range(2):
                        hh = 2 * p + hx
                        nc.tensor.matmul(
                            s2[:, 512 * hx : 512 * (hx + 1)],
                            kT_sb[g][32 * hh : 32 * (hh + 1), tt * 128 : (tt + 1) * 128],
                            qT_sb[g][32 * hh : 32 * (hh + 1), lh * 512 : (lh + 1) * 512],
                            start=True,
                            stop=True,
                            tile_position=(32 * hh, 0),
                        )
                    e = pools["e"].tile([128, 1024], BF16, tag="E", name=f"E{lh}{g}{p}{tt}")
                    nc.scalar.activation(e[:], s2[:], AF.Exp)
                    E[p][tt] = e
            if debug and (lh, g) == DBG_BLOCK:
                for tt in range(TT):
                    nc.sync.dma_start(dbg["E"][tt][:, 0:1024], E[0][tt][:])
                    nc.sync.dma_start(dbg["E"][tt][:, 1024:2048], E[1][tt][:])
            return E

        def pv_block(lh, g, E):
            """PV + normalize for (lh, g) consuming that block's E tiles."""
            ovs_g = []
            sg = ps["sg"].tile([128, 512], F32, tag="sg", name=f"sg{lh}{g}")
            for hl in range(4):
                ov = ps["ov"].tile([128, 512], F32, tag="ov", name=f"ov{lh}{g}{hl}")
                for tt in range(TT):
                    nc.tensor.matmul(
                        ov[0:34, :],
                        vaug[tt][:, 4 * g + hl, :],
                        E[hl // 2][tt][:, 512 * (hl % 2) : 512 * (hl % 2 + 1)],
                        start=(tt == 0),
                        stop=(tt == TT - 1),
                    )
                ovs = pools["ov"].tile([128, 512], F32R, tag="ovs", name=f"ovs{lh}{g}{hl}")
                nc.vector.tensor_copy(ovs[0:34, :], ov[0:34, :])
                ovs_g.append(ovs)
                # gather this head's sums row into partition 32*hl of sg
                nc.tensor.matmul(
                    sg[:],
                    ind4[hl][:],
                    ovs[0:34, :],
                    start=(hl == 0),
                    stop=(hl == 3),
                )
            rrf = pools["r"].tile([128, 512], F32, tag="rrf", name=f"rrf{lh}{g}")
            nc.vector.reciprocal_approx_fast(rrf[0:4, :], sg[0:4, :])
            rr = pools["r"].tile([128, 512], F32R, tag="rr", name=f"rr{lh}{g}")
            nc.vector.tensor_copy(rr[0:4, :], rrf[0:4, :])
            # one K=4 matmul broadcasts all 4 heads' recips to partitions
            # 32*hl .. 32*hl+31
            Rb = ps["rb"].tile([128, 512], F32, tag="Rb", name=f"Rb{lh}{g}")
            nc.tensor.matmul(
                Rb[:], bind[:], rr[0:4, :], start=True, stop=True
            )
            for hl in range(4):
                nc.vector.tensor_tensor(
                    attn_sb[g][32 * hl : 32 * (hl + 1), lh * 512 : (lh + 1) * 512],
                    ovs_g[hl][0:32, :].bitcast(F32),
                    Rb[32 * hl : 32 * (hl + 1), :],
                    ALU.mult,
                )

        # ---------------- schedule ----------------
        ph_conv.close()
        pools["e"] = top.enter_context(tc.tile_pool(name="epool", bufs=32))
        pools["r"] = top.enter_context(tc.tile_pool(name="rpool", bufs=2))
        pools["ov"] = top.enter_context(tc.tile_pool(name="ovpool", bufs=6))
        ps["qk"] = ph1.enter_context(tc.tile_pool(name="qk_ps", bufs=2, space="PSUM"))
        ps["v"] = ph1.enter_context(tc.tile_pool(name="v_ps", bufs=2, space="PSUM"))
        qk_proj(0)
        E00 = scores_block(0, 0)
        E10 = scores_block(1, 0)
        v_proj()
        qk_proj(1)
        qk_proj(2)
        ph1.close()
        ph2 = ExitStack()
        ps["ov"] = ph2.enter_context(tc.tile_pool(name="ov_ps", bufs=2, space="PSUM"))
        ps["sg"] = ph2.enter_context(tc.tile_pool(name="sg_ps", bufs=1, space="PSUM"))
        ps["rb"] = ph2.enter_context(tc.tile_pool(name="rb_ps", bufs=1, space="PSUM"))

        pv_block(0, 0, E00)
        E01 = scores_block(0, 1)
        pv_block(1, 0, E10)
        E11 = scores_block(1, 1)
        pv_block(0, 1, E01)
        E02 = scores_block(0, 2)
        pv_block(1, 1, E11)
        E12 = scores_block(1, 2)
        pv_block(0, 2, E02)
        pv_block(1, 2, E12)
        ph2.close()
        if debug:
            for i in range(CT):
                nc.sync.dma_start(dbg["qT"][i * 128 : (i + 1) * 128, :], qT_sb[i][:])
                nc.sync.dma_start(dbg["attn"][i * 128 : (i + 1) * 128, :], attn_sb[i][:])

        # ---------------- output projection ----------------
        with tc.tile_pool(name="o_ps", bufs=3, space="PSUM") as o_ps:
            for ot in range(CT):
                for th in range(TH):
                    op = o_ps.tile([128, 512], F32, tag="o", name=f"op{ot}{th}")
                    for kt in range(CT):
                        nc.tensor.matmul(
                            op[:],
                            wT_sb["o"][kt][:, ot * 128 : (ot + 1) * 128],
                            attn_sb[kt][:, th * 512 : (th + 1) * 512],
                            start=(kt == 0),
                            stop=(kt == CT - 1),
                        )
                    oc = copies.tile([128, 512], F32, tag="oc", name=f"oc{ot}{th}")
                    nc.vector.tensor_copy(oc[:], op[:])
                    nc.sync.dma_start(
                        outT_d[ot * 128 : (ot + 1) * 128, th * 512 : (th + 1) * 512],
                        oc[:],
                    )

    nc.compile()
    return nc


def _prep_inputs(x, conv_w, bn_gamma, bn_beta, bn_mean, bn_var, wq, wk, wv, wo):
    import ml_dtypes

    f32 = np.float32
    bf16 = ml_dtypes.bfloat16
    inv = (bn_gamma / np.sqrt(bn_var + BN_EPS)).astype(f32)
    w9 = (conv_w.reshape(C, 9) * inv[:, None]).astype(f32)
    bias = (bn_beta - bn_mean * inv).astype(f32).reshape(C, 1)
    wqT = np.ascontiguousarray((np.asarray(wq, f32) * f32(SCALE)).T).astype(bf16)
    wkT = np.ascontiguousarray(np.asarray(wk, f32).T).astype(bf16)
    wvT = np.ascontiguousarray(np.asarray(wv, f32).T).astype(bf16)
    woT = np.ascontiguousarray(np.asarray(wo, f32).T).astype(bf16)
    ind = np.zeros((4, 34, 128), f32)
    for j in range(4):
        ind[j, 32, j] = 1.0
    bind = np.zeros((4, 128), f32)
    for j in range(4):
        bind[j, 32 * j : 32 * (j + 1)] = 1.0
    maps = []
    for b in range(B):
        maps.append(
            {
                "xt": np.ascontiguousarray(np.asarray(x[b], f32).T).astype(bf16),
                "w9": w9,
                "bias": bias,
                "wqT": wqT,
                "wkT": wkT,
                "wvT": wvT,
                "woT": woT,
                "ind": ind,
                "bind": bind,
            }
        )
    return maps


def kernel(x, conv_w, bn_gamma, bn_beta, bn_mean, bn_var, wq, wk, wv, wo, h, w,
           **kw):
    assert int(h) == HH and int(w) == WW
    from concourse.bass_utils import run_bass_kernel_spmd

    if "nc" not in _CACHE:
        _CACHE["nc"] = _build()
    nc = _CACHE["nc"]
    maps = _prep_inputs(
        x, conv_w, bn_gamma, bn_beta, bn_mean, bn_var, wq, wk, wv, wo
    )
    res = run_bass_kernel_spmd(nc, maps, list(range(NCORES)))
    out = np.stack([res.results[b]["outT"].T for b in range(B)])
    return out.astype(np.float32)


# revision 36
# speedup vs baseline: 1.6578x; 1.2392x over previous
"""Trainium2 Bass kernel for conv-projected multi-head attention (v5).

Reference computation (per batch item b of 8, one NeuronCore each):
  y   = BN(depthwise3x3(x_b reshaped to [C,32,32]))      # q = k = v = y
  q/k/v = y @ w{q,k,v}^T  (heads: 12 x 32)
  att = softmax((q @ k^T) * sqrt(32))
  out = (att @ v) @ wo^T

v5 vs the fp32r baseline (263 us):
 - conv, q/k/v/out projections and the score matmuls run with bf16
   operands (psum stays f32); sqrt(32) folded into wq host-side.
   NOTE: exp with bf16 output miscompiles (writes raw f32) and walrus
   rejects mixed 32/16-bit matmul inputs, so E and vaug stay f32r and
   PV runs f32r like the baseline.
 - reciprocal_approx_fast replaces the 3.3us-per-call precise reciprocal
 - software-pipelined emission: scores+exp of block k+1 are emitted
   around PV of block k so ACT(exp) always has a backlog; attention for
   group 0 is emitted before the v projection / qk groups 1-2 so the
   scalar engine starts early.
Layout is channel-major: xT [C=384, T=1024] per core; S^T[t, l] per head;
vaug [t, h, 34] with a ones column so PV also yields the softmax
denominators in psum row 32 (gathered via ind4 matmuls, broadcast via a
K=4 bind matmul, exactly as the baseline).
"""
import sys

sys.path.insert(0, "/opt/trn_rl_repo")
from contextlib import ExitStack

import numpy as np

B, T, C = 8, 1024, 384
NH, DH = 12, 32
HH = WW = 32
SCALE = float(DH) ** 0.5
BN_EPS = 1e-5
NCORES = 8

_CACHE = {}


def _build(debug=False):
    import concourse.bass as bass
    import concourse.tile as tile
    from concourse import bacc, mybir
    from concourse.masks import make_identity

    F32 = mybir.dt.float32
    F32R = mybir.dt.float32r
    BF16 = mybir.dt.bfloat16
    AF = mybir.ActivationFunctionType
    ALU = mybir.AluOpType

    nc = bacc.Bacc("TRN2", target_bir_lowering=False, debug=False)

    xp_d = nc.dram_tensor("xp", [C, 34 * 34], BF16, kind="ExternalInput").ap()
    diag_d = nc.dram_tensor("diag", [C, 9, 128], BF16, kind="ExternalInput").ap()
    bias_d = nc.dram_tensor("bias", [C, 1], F32, kind="ExternalInput").ap()
    wqT_d = nc.dram_tensor("wqT", [C, C], BF16, kind="ExternalInput").ap()
    wkT_d = nc.dram_tensor("wkT", [C, C], BF16, kind="ExternalInput").ap()
    wvT_d = nc.dram_tensor("wvT", [C, C], BF16, kind="ExternalInput").ap()
    woT_d = nc.dram_tensor("woT", [C, C], BF16, kind="ExternalInput").ap()
    ind_d = nc.dram_tensor("ind", [4, 34, 128], F32R, kind="ExternalInput").ap()
    bind_d = nc.dram_tensor("bind", [4, 128], F32R, kind="ExternalInput").ap()
    outT_d = nc.dram_tensor("outT", [C, T], F32, kind="ExternalOutput").ap()
    dbg = {}
    if debug:
        dbg["y"] = nc.dram_tensor("dbg_y", [C, T], BF16, kind="ExternalOutput").ap()
        dbg["qT"] = nc.dram_tensor("dbg_qT", [C, T], BF16, kind="ExternalOutput").ap()
        dbg["attn"] = nc.dram_tensor(
            "dbg_attn", [C, T], BF16, kind="ExternalOutput"
        ).ap()
        dbg["E"] = nc.dram_tensor(
            "dbg_E", [8, 128, 2048], BF16, kind="ExternalOutput"
        ).ap()

    CT = C // 128  # 3 c-tiles / head groups of 4
    TT = T // 128  # 8 t-tiles
    TH = T // 512  # 2 l-halves
    DBG_BLOCK = (1, 0)  # (lh, g) block to dump in debug mode

    with tile.TileContext(nc) as tc, ExitStack() as top:
        persist = top.enter_context(tc.tile_pool(name="persist", bufs=1))
        copies = top.enter_context(tc.tile_pool(name="copies", bufs=3))

        y_sb = [persist.tile([128, T], BF16, tag=f"y{i}", name=f"y{i}") for i in range(CT)]
        qT_sb = [persist.tile([128, T], BF16, tag=f"q{i}", name=f"q{i}") for i in range(CT)]
        kT_sb = [persist.tile([128, T], BF16, tag=f"k{i}", name=f"k{i}") for i in range(CT)]
        vaug = [persist.tile([128, NH, 34], BF16, tag=f"va{i}", name=f"va{i}") for i in range(TT)]
        attn_sb = [persist.tile([128, T], BF16, tag=f"at{i}", name=f"at{i}") for i in range(CT)]

        # PSUM banks (8 x 2KB), pools strictly LIFO-nested per space:
        #   s_ps 4 (one s4 [128,2048], bufs=1) spans the whole kernel
        #   conv window:  s 4 + conv_ps 2                 = 6
        #   qk/v window:  s 4 + qk_ps 2 + v_ps 2          = 8
        #   attention:    s 4 + ov 2 + sg 1 + rb 1        = 8
        #   out-proj:     s 4 + o_ps 3                    = 7
        s_ps = top.enter_context(tc.tile_pool(name="s_ps", bufs=2, space="PSUM"))

        # ---------------- phase 1: conv ----------------
        ph_conv = ExitStack()
        convpool = ph_conv.enter_context(tc.tile_pool(name="convpool", bufs=1))
        conv_ps = ph_conv.enter_context(tc.tile_pool(name="conv_ps", bufs=2, space="PSUM"))
        ph1 = ExitStack()  # qk_ps/v_ps entered after conv pools close

        xp = [convpool.tile([128, 34 * 34], BF16, tag=f"xp{i}", name=f"xp{i}") for i in range(CT)]
        diag = [convpool.tile([128, 9, 128], BF16, tag=f"dg{i}", name=f"dg{i}") for i in range(CT)]

        for tt in range(TT):
            nc.gpsimd.memset(vaug[tt][:, :, 32:34], 0.0)
            nc.gpsimd.memset(vaug[tt][:, :, 32:33], 1.0)
        for i in range(CT):
            nc.sync.dma_start(xp[i][:], xp_d[i * 128 : (i + 1) * 128, :])
            nc.sync.dma_start(
                diag[i][:].rearrange("p a b -> p (a b)"),
                diag_d[i * 128 : (i + 1) * 128],
            )

        # weight / constant DMAs after the conv inputs so conv starts early
        wT_sb = {}
        for nm, d in (("q", wqT_d), ("k", wkT_d), ("v", wvT_d), ("o", woT_d)):
            tiles = [persist.tile([128, C], BF16, tag=f"w{nm}{i}", name=f"w{nm}{i}") for i in range(CT)]
            for i in range(CT):
                nc.sync.dma_start(tiles[i][:], d[i * 128 : (i + 1) * 128, :])
            wT_sb[nm] = tiles

        bias_sb = [persist.tile([128, 1], F32, tag=f"b{i}", name=f"b{i}") for i in range(CT)]
        for i in range(CT):
            nc.sync.dma_start(bias_sb[i][:], bias_d[i * 128 : (i + 1) * 128, :])
        ind4 = []
        for j in range(4):
            it = persist.tile([34, 128], F32R, tag=f"ind{j}", name=f"ind{j}")
            nc.sync.dma_start(it[:], ind_d[j])
            ind4.append(it)
        bind = persist.tile([4, 128], F32R, tag="bind", name="bind")
        nc.sync.dma_start(bind[:], bind_d)

        # conv: 9 accumulating diag matmuls per (c-tile, t-half)
        for i in range(CT):
            for th in range(TH):
                yp = conv_ps.tile([128, 512], F32, tag="conv", name=f"yp{i}{th}")
                r0 = th * 16
                for k in range(9):
                    dy, dx = k // 3 - 1, k % 3 - 1
                    off = (r0 + 1 + dy) * 34 + (1 + dx)
                    rhs = bass.AP(
                        tensor=xp[i].tensor,
                        offset=xp[i].offset + off,
                        ap=[list(p) for p in xp[i].ap[:1]] + [[34, 16], [1, 32]],
                    )
                    nc.tensor.matmul(
                        yp[:].rearrange("p (a b) -> p a b", a=16),
                        diag[i][:, k, :],
                        rhs,
                        start=(k == 0),
                        stop=(k == 8),
                    )
                nc.vector.tensor_scalar_add(
                    y_sb[i][:, th * 512 : (th + 1) * 512], yp[:], bias_sb[i][:]
                )
        if debug:
            for i in range(CT):
                nc.sync.dma_start(dbg["y"][i * 128 : (i + 1) * 128, :], y_sb[i][:])

        ps = {}
        pools = {}

        def qk_proj(ot):
            for nm, dst in (("q", qT_sb), ("k", kT_sb)):
                for th in range(TH):
                    pp = ps["qk"].tile([128, 512], F32, tag="qk", name=f"pp{nm}{ot}{th}")
                    for kt in range(CT):
                        nc.tensor.matmul(
                            pp[:],
                            wT_sb[nm][kt][:, ot * 128 : (ot + 1) * 128],
                            y_sb[kt][:, th * 512 : (th + 1) * 512],
                            start=(kt == 0),
                            stop=(kt == CT - 1),
                        )
                    nc.vector.tensor_copy(dst[ot][:, th * 512 : (th + 1) * 512], pp[:])

        def v_proj():
            for tt in range(TT):
                vp = ps["v"].tile([128, C], F32, tag="v", name=f"vp{tt}")
                for kt in range(CT):
                    nc.tensor.matmul(
                        vp[:],
                        y_sb[kt][:, tt * 128 : (tt + 1) * 128],
                        wT_sb["v"][kt][:],
                        start=(kt == 0),
                        stop=(kt == CT - 1),
                    )
                nc.vector.tensor_copy(
                    vaug[tt][:, :, 0:32], vp[:].rearrange("p (h d) -> p h d", h=NH)
                )

        def scores_block(lh, g):
            """Scores + exp for (l-half lh, head-group g). Returns E[p][tt]
            tiles, each [128, 1024] covering head pair (2p, 2p+1)."""
            E = [[None] * TT for _ in range(2)]
            for p in range(2):
                for tt in range(TT):
                    s2 = s_ps.tile([128, 1024], F32, tag="s2", name=f"s{lh}{g}{p}{tt}")
                    for hx in range(2):
                        hh = 2 * p + hx
                        nc.tensor.matmul(
                            s2[:, 512 * hx : 512 * (hx + 1)],
                            kT_sb[g][32 * hh : 32 * (hh + 1), tt * 128 : (tt + 1) * 128],
                            qT_sb[g][32 * hh : 32 * (hh + 1), lh * 512 : (lh + 1) * 512],
                            start=True,
                            stop=True,
                            tile_position=(32 * hh, 0),
                        )
                    e = pools["e"].tile([128, 1024], BF16, tag="E", name=f"E{lh}{g}{p}{tt}")
                    nc.scalar.activation(e[:], s2[:], AF.Exp)
                    E[p][tt] = e
            if debug and (lh, g) == DBG_BLOCK:
                for tt in range(TT):
                    nc.sync.dma_start(dbg["E"][tt][:, 0:1024], E[0][tt][:])
                    nc.sync.dma_start(dbg["E"][tt][:, 1024:2048], E[1][tt][:])
            return E

        def pv_block(lh, g, E):
            """PV + normalize for (lh, g) consuming that block's E tiles."""
            ovs_g = []
            sg = ps["sg"].tile([128, 512], F32, tag="sg", name=f"sg{lh}{g}")
            for hl in range(4):
                ov = ps["ov"].tile([128, 512], F32, tag="ov", name=f"ov{lh}{g}{hl}")
                for tt in range(TT):
                    nc.tensor.matmul(
                        ov[0:34, :],
                        vaug[tt][:, 4 * g + hl, :],
                        E[hl // 2][tt][:, 512 * (hl % 2) : 512 * (hl % 2 + 1)],
                        start=(tt == 0),
                        stop=(tt == TT - 1),
                    )
                ovs = pools["ov"].tile([128, 512], F32R, tag="ovs", name=f"ovs{lh}{g}{hl}")
                nc.vector.tensor_copy(ovs[0:34, :], ov[0:34, :])
                ovs_g.append(ovs)
                # gather this head's sums row into partition 32*hl of sg
                nc.tensor.matmul(
                    sg[:],
                    ind4[hl][:],
                    ovs[0:34, :],
                    start=(hl == 0),
                    stop=(hl == 3),
                )
            rrf = pools["r"].tile([128, 512], F32, tag="rrf", name=f"rrf{lh}{g}")
            nc.vector.reciprocal_approx_fast(rrf[0:4, :], sg[0:4, :])
            rr = pools["r"].tile([128, 512], F32R, tag="rr", name=f"rr{lh}{g}")
            nc.vector.tensor_copy(rr[0:4, :], rrf[0:4, :])
            # one K=4 matmul broadcasts all 4 heads' recips to partitions
            # 32*hl .. 32*hl+31
            Rb = ps["rb"].tile([128, 512], F32, tag="Rb", name=f"Rb{lh}{g}")
            nc.tensor.matmul(
                Rb[:], bind[:], rr[0:4, :], start=True, stop=True
            )
            for hl in range(4):
                nc.vector.tensor_tensor(
                    attn_sb[g][32 * hl : 32 * (hl + 1), lh * 512 : (lh + 1) * 512],
                    ovs_g[hl][0:32, :].bitcast(F32),
                    Rb[32 * hl : 32 * (hl + 1), :],
                    ALU.mult,
                )

        # ---------------- schedule ----------------
        ph_conv.close()
        pools["e"] = top.enter_context(tc.tile_pool(name="epool", bufs=32))
        pools["r"] = top.enter_context(tc.tile_pool(name="rpool", bufs=2))
        pools["ov"] = top.enter_context(tc.tile_pool(name="ovpool", bufs=6))
        ps["qk"] = ph1.enter_context(tc.tile_pool(name="qk_ps", bufs=2, space="PSUM"))
        ps["v"] = ph1.enter_context(tc.tile_pool(name="v_ps", bufs=2, space="PSUM"))
        qk_proj(0)
        E00 = scores_block(0, 0)
        E10 = scores_block(1, 0)
        v_proj()
        qk_proj(1)
        qk_proj(2)
        ph1.close()
        ph2 = ExitStack()
        ps["ov"] = ph2.enter_context(tc.tile_pool(name="ov_ps", bufs=2, space="PSUM"))
        ps["sg"] = ph2.enter_context(tc.tile_pool(name="sg_ps", bufs=1, space="PSUM"))
        ps["rb"] = ph2.enter_context(tc.tile_pool(name="rb_ps", bufs=1, space="PSUM"))

        pv_block(0, 0, E00)
        E01 = scores_block(0, 1)
        pv_block(1, 0, E10)
        E11 = scores_block(1, 1)
        pv_block(0, 1, E01)
        E02 = scores_block(0, 2)
        pv_block(1, 1, E11)
        E12 = scores_block(1, 2)
        pv_block(0, 2, E02)
        pv_block(1, 2, E12)
        ph2.close()
        if debug:
            for i in range(CT):
                nc.sync.dma_start(dbg["qT"][i * 128 : (i + 1) * 128, :], qT_sb[i][:])
                nc.sync.dma_start(dbg["attn"][i * 128 : (i + 1) * 128, :], attn_sb[i][:])

        # ---------------- output projection ----------------
        with tc.tile_pool(name="o_ps", bufs=3, space="PSUM") as o_ps:
            for ot in range(CT):
                for th in range(TH):
                    op = o_ps.tile([128, 512], F32, tag="o", name=f"op{ot}{th}")
                    for kt in range(CT):
                        nc.tensor.matmul(
                            op[:],
                            wT_sb["o"][kt][:, ot * 128 : (ot + 1) * 128],
                            attn_sb[kt][:, th * 512 : (th + 1) * 512],
                            start=(kt == 0),
                            stop=(kt == CT - 1),
                        )
                    oc = copies.tile([128, 512], F32, tag="oc", name=f"oc{ot}{th}")
                    nc.vector.tensor_copy(oc[:], op[:])
                    nc.sync.dma_start(
                        outT_d[ot * 128 : (ot + 1) * 128, th * 512 : (th + 1) * 512],
                        oc[:],
                    )

    nc.compile()
    return nc


def _prep_inputs(x, conv_w, bn_gamma, bn_beta, bn_mean, bn_var, wq, wk, wv, wo):
    import ml_dtypes

    f32 = np.float32
    bf16 = ml_dtypes.bfloat16
    inv = (bn_gamma / np.sqrt(bn_var + BN_EPS)).astype(f32)
    w9 = (conv_w.reshape(C, 9) * inv[:, None]).astype(f32)
    bias = (bn_beta - bn_mean * inv).astype(f32).reshape(C, 1)
    diag = np.zeros((C, 9, 128), f32)
    cc = np.arange(C)
    diag[cc[:, None], np.arange(9)[None, :], (cc % 128)[:, None]] = w9
    diag = diag.astype(bf16)
    wqT = np.ascontiguousarray((np.asarray(wq, f32) * f32(SCALE)).T).astype(bf16)
    wkT = np.ascontiguousarray(np.asarray(wk, f32).T).astype(bf16)
    wvT = np.ascontiguousarray(np.asarray(wv, f32).T).astype(bf16)
    woT = np.ascontiguousarray(np.asarray(wo, f32).T).astype(bf16)
    ind = np.zeros((4, 34, 128), f32)
    for j in range(4):
        ind[j, 32, j] = 1.0
    bind = np.zeros((4, 128), f32)
    for j in range(4):
        bind[j, 32 * j : 32 * (j + 1)] = 1.0
    maps = []
    for b in range(B):
        xt = np.ascontiguousarray(np.asarray(x[b], f32).T)
        xpad = np.zeros((C, 34, 34), f32)
        xpad[:, 1:33, 1:33] = xt.reshape(C, 32, 32)
        maps.append(
            {
                "xp": xpad.reshape(C, 34 * 34).astype(bf16),
                "diag": diag,
                "bias": bias,
                "wqT": wqT,
                "wkT": wkT,
                "wvT": wvT,
                "woT": woT,
                "ind": ind,
                "bind": bind,
            }
        )
    return maps


def kernel(x, conv_w, bn_gamma, bn_beta, bn_mean, bn_var, wq, wk, wv, wo, h, w,
           **kw):
    assert int(h) == HH and int(w) == WW
    from concourse.bass_utils import run_bass_kernel_spmd

    if "nc" not in _CACHE:
        _CACHE["nc"] = _build()
    nc = _CACHE["nc"]
    maps = _prep_inputs(
        x, conv_w, bn_gamma, bn_beta, bn_mean, bn_var, wq, wk, wv, wo
    )
    res = run_bass_kernel_spmd(nc, maps, list(range(NCORES)))
    out = np.stack([res.results[b]["outT"].T for b in range(B)])
    return out.astype(np.float32)


# revision 38
# speedup vs baseline: 1.7165x; 1.0354x over previous
"""Trainium2 Bass kernel for conv-projected multi-head attention (v5).

Reference computation (per batch item b of 8, one NeuronCore each):
  y   = BN(depthwise3x3(x_b reshaped to [C,32,32]))      # q = k = v = y
  q/k/v = y @ w{q,k,v}^T  (heads: 12 x 32)
  att = softmax((q @ k^T) * sqrt(32))
  out = (att @ v) @ wo^T

v5 vs the fp32r baseline (263 us):
 - conv, q/k/v/out projections and the score matmuls run with bf16
   operands (psum stays f32); sqrt(32) folded into wq host-side.
   NOTE: exp with bf16 output miscompiles (writes raw f32) and walrus
   rejects mixed 32/16-bit matmul inputs, so E and vaug stay f32r and
   PV runs f32r like the baseline.
 - reciprocal_approx_fast replaces the 3.3us-per-call precise reciprocal
 - software-pipelined emission: scores+exp of block k+1 are emitted
   around PV of block k so ACT(exp) always has a backlog; attention for
   group 0 is emitted before the v projection / qk groups 1-2 so the
   scalar engine starts early.
Layout is channel-major: xT [C=384, T=1024] per core; S^T[t, l] per head;
vaug [t, h, 34] with a ones column so PV also yields the softmax
denominators in psum row 32 (gathered via ind4 matmuls, broadcast via a
K=4 bind matmul, exactly as the baseline).
"""
import sys

sys.path.insert(0, "/opt/trn_rl_repo")
from contextlib import ExitStack

import numpy as np

B, T, C = 8, 1024, 384
NH, DH = 12, 32
HH = WW = 32
SCALE = float(DH) ** 0.5
BN_EPS = 1e-5
NCORES = 8

_CACHE = {}


def _build(debug=False):
    import concourse.bass as bass
    import concourse.tile as tile
    from concourse import bacc, mybir
    from concourse.masks import make_identity

    F32 = mybir.dt.float32
    F32R = mybir.dt.float32r
    BF16 = mybir.dt.bfloat16
    AF = mybir.ActivationFunctionType
    ALU = mybir.AluOpType

    nc = bacc.Bacc("TRN2", target_bir_lowering=False, debug=False)

    xp_d = nc.dram_tensor("xp", [C, 34 * 34], BF16, kind="ExternalInput").ap()
    diag_d = nc.dram_tensor("diag", [C, 9, 128], BF16, kind="ExternalInput").ap()
    bias_d = nc.dram_tensor("bias", [C, 1], F32, kind="ExternalInput").ap()
    wqT_d = nc.dram_tensor("wqT", [C, C], BF16, kind="ExternalInput").ap()
    wkT_d = nc.dram_tensor("wkT", [C, C], BF16, kind="ExternalInput").ap()
    wvT_d = nc.dram_tensor("wvT", [C, C], BF16, kind="ExternalInput").ap()
    woT_d = nc.dram_tensor("woT", [C, C], BF16, kind="ExternalInput").ap()
    ind_d = nc.dram_tensor("ind", [4, 34, 128], F32R, kind="ExternalInput").ap()
    bind_d = nc.dram_tensor("bind", [4, 128], F32R, kind="ExternalInput").ap()
    outT_d = nc.dram_tensor("outT", [C, T], F32, kind="ExternalOutput").ap()
    dbg = {}
    if debug:
        dbg["y"] = nc.dram_tensor("dbg_y", [C, T], BF16, kind="ExternalOutput").ap()
        dbg["qT"] = nc.dram_tensor("dbg_qT", [C, T], BF16, kind="ExternalOutput").ap()
        dbg["attn"] = nc.dram_tensor(
            "dbg_attn", [C, T], BF16, kind="ExternalOutput"
        ).ap()
        dbg["E"] = nc.dram_tensor(
            "dbg_E", [8, 128, 2048], BF16, kind="ExternalOutput"
        ).ap()

    CT = C // 128  # 3 c-tiles / head groups of 4
    TT = T // 128  # 8 t-tiles
    TH = T // 512  # 2 l-halves
    DBG_BLOCK = (1, 0)  # (lh, g) block to dump in debug mode

    with tile.TileContext(nc) as tc, ExitStack() as top:
        persist = top.enter_context(tc.tile_pool(name="persist", bufs=1))
        copies = top.enter_context(tc.tile_pool(name="copies", bufs=3))

        y_sb = [persist.tile([128, T], BF16, tag=f"y{i}", name=f"y{i}") for i in range(CT)]
        qT_sb = [persist.tile([128, T], BF16, tag=f"q{i}", name=f"q{i}") for i in range(CT)]
        kT_sb = [persist.tile([128, T], BF16, tag=f"k{i}", name=f"k{i}") for i in range(CT)]
        vaug = [persist.tile([128, NH, 34], BF16, tag=f"va{i}", name=f"va{i}") for i in range(TT)]
        attn_sb = [persist.tile([128, T], BF16, tag=f"at{i}", name=f"at{i}") for i in range(CT)]

        # PSUM banks (8 x 2KB), pools strictly LIFO-nested per space:
        #   s_ps 4 (one s4 [128,2048], bufs=1) spans the whole kernel
        #   conv window:  s 4 + conv_ps 2                 = 6
        #   qk/v window:  s 4 + qk_ps 2 + v_ps 2          = 8
        #   attention:    s 4 + ov 2 + sg 1 + rb 1        = 8
        #   out-proj:     s 4 + o_ps 3                    = 7
        s_ps = top.enter_context(tc.tile_pool(name="s_ps", bufs=2, space="PSUM"))

        # ---------------- phase 1: conv ----------------
        ph_conv = ExitStack()
        convpool = ph_conv.enter_context(tc.tile_pool(name="convpool", bufs=1))
        conv_ps = ph_conv.enter_context(tc.tile_pool(name="conv_ps", bufs=2, space="PSUM"))
        ph1 = ExitStack()  # qk_ps/v_ps entered after conv pools close

        xp = [convpool.tile([128, 34 * 34], BF16, tag=f"xp{i}", name=f"xp{i}") for i in range(CT)]
        diag = [convpool.tile([128, 9, 128], BF16, tag=f"dg{i}", name=f"dg{i}") for i in range(CT)]

        for tt in range(TT):
            nc.gpsimd.memset(vaug[tt][:, :, 32:34], 0.0)
            nc.gpsimd.memset(vaug[tt][:, :, 32:33], 1.0)
        for i in range(CT):
            nc.sync.dma_start(xp[i][:], xp_d[i * 128 : (i + 1) * 128, :])
            nc.sync.dma_start(
                diag[i][:].rearrange("p a b -> p (a b)"),
                diag_d[i * 128 : (i + 1) * 128],
            )

        # bias + weights on the vector queue, ind/bind on gpsimd: the sync
        # queue then carries only the 6 conv-input DMAs, so conv starts early
        # and its bias-add never waits behind the weight transfers.
        bias_sb = [persist.tile([128, 1], F32, tag=f"b{i}", name=f"b{i}") for i in range(CT)]
        for i in range(CT):
            nc.scalar.dma_start(bias_sb[i][:], bias_d[i * 128 : (i + 1) * 128, :])
        wT_sb = {}
        for nm, d in (("q", wqT_d), ("k", wkT_d), ("v", wvT_d), ("o", woT_d)):
            tiles = [persist.tile([128, C], BF16, tag=f"w{nm}{i}", name=f"w{nm}{i}") for i in range(CT)]
            for i in range(CT):
                nc.scalar.dma_start(tiles[i][:], d[i * 128 : (i + 1) * 128, :])
            wT_sb[nm] = tiles
        ind4 = []
        for j in range(4):
            it = persist.tile([34, 128], F32R, tag=f"ind{j}", name=f"ind{j}")
            nc.gpsimd.dma_start(it[:], ind_d[j])
            ind4.append(it)
        bind = persist.tile([4, 128], F32R, tag="bind", name="bind")
        nc.gpsimd.dma_start(bind[:], bind_d)

        # conv: 9 accumulating diag matmuls per (c-tile, t-half)
        for i in range(CT):
            for th in range(TH):
                yp = conv_ps.tile([128, 512], F32, tag="conv", name=f"yp{i}{th}")
                r0 = th * 16
                for k in range(9):
                    dy, dx = k // 3 - 1, k % 3 - 1
                    off = (r0 + 1 + dy) * 34 + (1 + dx)
                    rhs = bass.AP(
                        tensor=xp[i].tensor,
                        offset=xp[i].offset + off,
                        ap=[list(p) for p in xp[i].ap[:1]] + [[34, 16], [1, 32]],
                    )
                    nc.tensor.matmul(
                        yp[:].rearrange("p (a b) -> p a b", a=16),
                        diag[i][:, k, :],
                        rhs,
                        start=(k == 0),
                        stop=(k == 8),
                    )
                nc.vector.tensor_scalar_add(
                    y_sb[i][:, th * 512 : (th + 1) * 512], yp[:], bias_sb[i][:]
                )
        if debug:
            for i in range(CT):
                nc.sync.dma_start(dbg["y"][i * 128 : (i + 1) * 128, :], y_sb[i][:])

        ps = {}
        pools = {}

        def qk_proj(ot):
            for nm, dst in (("q", qT_sb), ("k", kT_sb)):
                for th in range(TH):
                    pp = ps["qk"].tile([128, 512], F32, tag="qk", name=f"pp{nm}{ot}{th}")
                    for kt in range(CT):
                        nc.tensor.matmul(
                            pp[:],
                            wT_sb[nm][kt][:, ot * 128 : (ot + 1) * 128],
                            y_sb[kt][:, th * 512 : (th + 1) * 512],
                            start=(kt == 0),
                            stop=(kt == CT - 1),
                        )
                    nc.vector.tensor_copy(dst[ot][:, th * 512 : (th + 1) * 512], pp[:])

        def v_proj():
            for tt in range(TT):
                vp = ps["v"].tile([128, C], F32, tag="v", name=f"vp{tt}")
                for kt in range(CT):
                    nc.tensor.matmul(
                        vp[:],
                        y_sb[kt][:, tt * 128 : (tt + 1) * 128],
                        wT_sb["v"][kt][:],
                        start=(kt == 0),
                        stop=(kt == CT - 1),
                    )
                nc.vector.tensor_copy(
                    vaug[tt][:, :, 0:32], vp[:].rearrange("p (h d) -> p h d", h=NH)
                )

        def scores_block(lh, g):
            """Scores + exp for (l-half lh, head-group g). Returns E[p][tt]
            tiles, each [128, 1024] covering head pair (2p, 2p+1)."""
            E = [[None] * TT for _ in range(2)]
            for p in range(2):
                for tt in range(TT):
                    s2 = s_ps.tile([128, 1024], F32, tag="s2", name=f"s{lh}{g}{p}{tt}")
                    for hx in range(2):
                        hh = 2 * p + hx
                        nc.tensor.matmul(
                            s2[:, 512 * hx : 512 * (hx + 1)],
                            kT_sb[g][32 * hh : 32 * (hh + 1), tt * 128 : (tt + 1) * 128],
                            qT_sb[g][32 * hh : 32 * (hh + 1), lh * 512 : (lh + 1) * 512],
                            start=True,
                            stop=True,
                            tile_position=(32 * hh, 0),
                        )
                    e = pools["e"].tile([128, 1024], BF16, tag="E", name=f"E{lh}{g}{p}{tt}")
                    nc.scalar.activation(e[:], s2[:], AF.Exp)
                    E[p][tt] = e
            if debug and (lh, g) == DBG_BLOCK:
                for tt in range(TT):
                    nc.sync.dma_start(dbg["E"][tt][:, 0:1024], E[0][tt][:])
                    nc.sync.dma_start(dbg["E"][tt][:, 1024:2048], E[1][tt][:])
            return E

        def pv_block(lh, g, E):
            """PV + normalize for (lh, g) consuming that block's E tiles."""
            ovs_g = []
            sg = ps["sg"].tile([128, 512], F32, tag="sg", name=f"sg{lh}{g}")
            for hl in range(4):
                ov = ps["ov"].tile([128, 512], F32, tag="ov", name=f"ov{lh}{g}{hl}")
                for tt in range(TT):
                    nc.tensor.matmul(
                        ov[0:34, :],
                        vaug[tt][:, 4 * g + hl, :],
                        E[hl // 2][tt][:, 512 * (hl % 2) : 512 * (hl % 2 + 1)],
                        start=(tt == 0),
                        stop=(tt == TT - 1),
                    )
                ovs = pools["ov"].tile([128, 512], F32R, tag="ovs", name=f"ovs{lh}{g}{hl}")
                nc.vector.tensor_copy(ovs[0:34, :], ov[0:34, :])
                ovs_g.append(ovs)
                # gather this head's sums row into partition 32*hl of sg
                nc.tensor.matmul(
                    sg[:],
                    ind4[hl][:],
                    ovs[0:34, :],
                    start=(hl == 0),
                    stop=(hl == 3),
                )
            rrf = pools["r"].tile([128, 512], F32, tag="rrf", name=f"rrf{lh}{g}")
            nc.vector.reciprocal_approx_fast(rrf[0:4, :], sg[0:4, :])
            rr = pools["r"].tile([128, 512], F32R, tag="rr", name=f"rr{lh}{g}")
            nc.vector.tensor_copy(rr[0:4, :], rrf[0:4, :])
            # one K=4 matmul broadcasts all 4 heads' recips to partitions
            # 32*hl .. 32*hl+31
            Rb = ps["rb"].tile([128, 512], F32, tag="Rb", name=f"Rb{lh}{g}")
            nc.tensor.matmul(
                Rb[:], bind[:], rr[0:4, :], start=True, stop=True
            )
            for hl in range(4):
                nc.vector.tensor_tensor(
                    attn_sb[g][32 * hl : 32 * (hl + 1), lh * 512 : (lh + 1) * 512],
                    ovs_g[hl][0:32, :].bitcast(F32),
                    Rb[32 * hl : 32 * (hl + 1), :],
                    ALU.mult,
                )

        # ---------------- schedule ----------------
        ph_conv.close()
        pools["e"] = top.enter_context(tc.tile_pool(name="epool", bufs=32))
        pools["r"] = top.enter_context(tc.tile_pool(name="rpool", bufs=2))
        pools["ov"] = top.enter_context(tc.tile_pool(name="ovpool", bufs=6))
        ps["qk"] = ph1.enter_context(tc.tile_pool(name="qk_ps", bufs=2, space="PSUM"))
        ps["v"] = ph1.enter_context(tc.tile_pool(name="v_ps", bufs=2, space="PSUM"))
        qk_proj(0)
        E00 = scores_block(0, 0)
        E10 = scores_block(1, 0)
        v_proj()
        qk_proj(1)
        qk_proj(2)
        ph1.close()
        ph2 = ExitStack()
        ps["ov"] = ph2.enter_context(tc.tile_pool(name="ov_ps", bufs=2, space="PSUM"))
        ps["sg"] = ph2.enter_context(tc.tile_pool(name="sg_ps", bufs=1, space="PSUM"))
        ps["rb"] = ph2.enter_context(tc.tile_pool(name="rb_ps", bufs=1, space="PSUM"))

        pv_block(0, 0, E00)
        E01 = scores_block(0, 1)
        pv_block(1, 0, E10)
        E11 = scores_block(1, 1)
        pv_block(0, 1, E01)
        E02 = scores_block(0, 2)
        pv_block(1, 1, E11)
        E12 = scores_block(1, 2)
        pv_block(0, 2, E02)
        pv_block(1, 2, E12)
        ph2.close()
        if debug:
            for i in range(CT):
                nc.sync.dma_start(dbg["qT"][i * 128 : (i + 1) * 128, :], qT_sb[i][:])
                nc.sync.dma_start(dbg["attn"][i * 128 : (i + 1) * 128, :], attn_sb[i][:])

        # ---------------- output projection ----------------
        with tc.tile_pool(name="o_ps", bufs=3, space="PSUM") as o_ps:
            for ot in range(CT):
                for th in range(TH):
                    op = o_ps.tile([128, 512], F32, tag="o", name=f"op{ot}{th}")
                    for kt in range(CT):
                        nc.tensor.matmul(
                            op[:],
                            wT_sb["o"][kt][:, ot * 128 : (ot + 1) * 128],
                            attn_sb[kt][:, th * 512 : (th + 1) * 512],
                            start=(kt == 0),
                            stop=(kt == CT - 1),
                        )
                    oc = copies.tile([128, 512], F32, tag="oc", name=f"oc{ot}{th}")
                    nc.vector.tensor_copy(oc[:], op[:])
                    nc.sync.dma_start(
                        outT_d[ot * 128 : (ot + 1) * 128, th * 512 : (th + 1) * 512],
                        oc[:],
                    )

    nc.compile()
    return nc


def _prep_inputs(x, conv_w, bn_gamma, bn_beta, bn_mean, bn_var, wq, wk, wv, wo):
    import ml_dtypes

    f32 = np.float32
    bf16 = ml_dtypes.bfloat16
    inv = (bn_gamma / np.sqrt(bn_var + BN_EPS)).astype(f32)
    w9 = (conv_w.reshape(C, 9) * inv[:, None]).astype(f32)
    bias = (bn_beta - bn_mean * inv).astype(f32).reshape(C, 1)
    diag = np.zeros((C, 9, 128), f32)
    cc = np.arange(C)
    diag[cc[:, None], np.arange(9)[None, :], (cc % 128)[:, None]] = w9
    diag = diag.astype(bf16)
    wqT = np.ascontiguousarray((np.asarray(wq, f32) * f32(SCALE)).T).astype(bf16)
    wkT = np.ascontiguousarray(np.asarray(wk, f32).T).astype(bf16)
    wvT = np.ascontiguousarray(np.asarray(wv, f32).T).astype(bf16)
    woT = np.ascontiguousarray(np.asarray(wo, f32).T).astype(bf16)
    ind = np.zeros((4, 34, 128), f32)
    for j in range(4):
        ind[j, 32, j] = 1.0
    bind = np.zeros((4, 128), f32)
    for j in range(4):
        bind[j, 32 * j : 32 * (j + 1)] = 1.0
    maps = []
    for b in range(B):
        xt = np.ascontiguousarray(np.asarray(x[b], f32).T)
        xpad = np.zeros((C, 34, 34), f32)
        xpad[:, 1:33, 1:33] = xt.reshape(C, 32, 32)
        maps.append(
            {
                "xp": xpad.reshape(C, 34 * 34).astype(bf16),
                "diag": diag,
                "bias": bias,
                "wqT": wqT,
                "wkT": wkT,
                "wvT": wvT,
                "woT": woT,
                "ind": ind,
                "bind": bind,
            }
        )
    return maps


def kernel(x, conv_w, bn_gamma, bn_beta, bn_mean, bn_var, wq, wk, wv, wo, h, w,
           **kw):
    assert int(h) == HH and int(w) == WW
    from concourse.bass_utils import run_bass_kernel_spmd

    if "nc" not in _CACHE:
        _CACHE["nc"] = _build()
    nc = _CACHE["nc"]
    maps = _prep_inputs(
        x, conv_w, bn_gamma, bn_beta, bn_mean, bn_var, wq, wk, wv, wo
    )
    res = run_bass_kernel_spmd(nc, maps, list(range(NCORES)))
    out = np.stack([res.results[b]["outT"].T for b in range(B)])
    return out.astype(np.float32)
